# revision 16
# baseline (speedup 1.0000x reference)
"""CWRNN language-model kernel for 8 Trainium2 NeuronCores.

Strategy (vocab-sharded output projection, v2):
  - Each core owns Wo[:, c*4000:(c+1)*4000] and writes its logits slice in
    fp16 (tolerance 2e-2 >> fp16 rounding) -> halves the HBM write volume
    that dominated v1.
  - The clockwork mask is block-triangular: block0 (period 1, units 0:64)
    is fed only by itself, so the serial critical path is a 64-unit RNN.
    Blocks 1-3 run as separate serial chains lagging block0, with their
    cross-block input terms batched as per-tile matmuls over the recorded
    history tiles.
  - U = emb @ Wi is accumulated directly into PSUM banks (phase A); chain
    matmuls accumulate h @ Whh on top (start=False) and tanh reads PSUM
    with the bias folded into the activation -> each chain step is exactly
    matmul -> tanh, and tanh writes straight into the fp16 history tile
    column the next step's matmul reads (no DVE add, no Pool copy on the
    critical path).
  - Projection pairs two 500-col chunks per weight load, paced one unit
    per chain step two tiles behind the recurrence; all PSUM drains stay
    on DVE (ACT is reserved for the serial tanh chain; gpsimd has no PSUM
    access); one fp16 output DMA per tile on the sync queue.
  - Pacing note: the PE sustains only ~50% of its 2.4GHz peak (power
    throttle).  Denser projection pacing (more PSUM banks, bigger bursts)
    measurably LOWERS throughput once the throttle engages, so the
    drain-coupled pp=2 pacing is deliberate.
"""

import sys

sys.path.insert(0, "/opt/trn_rl_repo")

import numpy as np

import concourse.bass as bass
import concourse.mybir as mybir
import concourse.tile as tile
from concourse import bacc
from concourse import bass_utils as _bass_utils
from concourse.bass_utils import run_bass_kernel_spmd
from concourse.masks import make_identity

# note: walrus's --enable-ldw-opt pass crashes codegen on this toolchain
# (visitInstLdweights), so per-matmul LDWEIGHTS reloads are unavoidable

F32 = mybir.dt.float32
F16 = mybir.dt.float16
I32 = mybir.dt.int32
TANH = mybir.ActivationFunctionType.Tanh

B = 16
T = 255           # x[:, :-1]
E = 256
NH = 256
V = 32000
NCORES = 8
VS = V // NCORES  # 4000 vocab columns per core
NT = 32           # token tiles of 8 steps (tile 31 has 7 real steps)
VC = 500          # vocab chunk per PSUM bank
PROJ_LAG = 2      # tiles between recurrence and projection start


def build_program():
    nc = bacc.Bacc(target_bir_lowering=False)

    d_ids = nc.dram_tensor("tok_ids", [128, NT], I32, kind="ExternalInput")
    d_emb = nc.dram_tensor("embedding", [V + 1, E], F32, kind="ExternalInput")
    d_whA = nc.dram_tensor("whA", [128, 128], F16, kind="ExternalInput")
    d_wB2 = nc.dram_tensor("wB2", [128, 64], F16, kind="ExternalInput")
    d_wB3 = nc.dram_tensor("wB3", [128, 64], F16, kind="ExternalInput")
    d_wX2 = nc.dram_tensor("wX2", [128, 64], F16, kind="ExternalInput")
    d_wX3 = nc.dram_tensor("wX3", [128, 64], F16, kind="ExternalInput")
    d_wiA = nc.dram_tensor("wiA", [256, 128], F16, kind="ExternalInput")
    d_wiB = nc.dram_tensor("wiB", [256, 128], F16, kind="ExternalInput")
    d_biasA = nc.dram_tensor("biasA", [128, 1], F32, kind="ExternalInput")
    d_biasB = nc.dram_tensor("biasB", [128, 1], F32, kind="ExternalInput")
    d_wo0 = nc.dram_tensor("wo0", [128, VS], F16, kind="ExternalInput")
    d_wo1 = nc.dram_tensor("wo1", [128, VS], F16, kind="ExternalInput")
    # raw token-major output: row g*128 + b*8 + c  <->  logits[b, g*8+c]
    d_out = nc.dram_tensor("out", [NT * 128, VS], F16, kind="ExternalOutput")
    d_dbg = None
    if DEBUG_HT:
        d_dbg = nc.dram_tensor("dbg_ht", [NT * 128, 256], F16,
                               kind="ExternalOutput")

    with tile.TileContext(nc) as tc:
        with tc.tile_pool(name="const", bufs=1) as cpool, \
             tc.tile_pool(name="hist", bufs=1) as hpool, \
             tc.tile_pool(name="obuf", bufs=3) as opool, \
             tc.tile_pool(name="work", bufs=3) as wpool, \
             tc.tile_pool(name="psum", bufs=2, space="PSUM") as psum:

            # ---------------- constants and weights ----------------
            ids_sb = cpool.tile([128, NT], I32)
            nc.sync.dma_start(out=ids_sb[:], in_=d_ids[:])

            whA = cpool.tile([128, 128], F16, name="whA")
            nc.sync.dma_start(out=whA[:], in_=d_whA[:])
            wB2 = cpool.tile([128, 64], F16, name="wB2")
            nc.sync.dma_start(out=wB2[:], in_=d_wB2[:])
            wB3 = cpool.tile([128, 64], F16, name="wB3")
            nc.sync.dma_start(out=wB3[:], in_=d_wB3[:])
            wX2 = cpool.tile([128, 64], F16, name="wX2")
            nc.sync.dma_start(out=wX2[:], in_=d_wX2[:])
            wX3 = cpool.tile([128, 64], F16, name="wX3")
            nc.sync.dma_start(out=wX3[:], in_=d_wX3[:])
            wiA = [cpool.tile([128, 128], F16, name=f"wiA{k}") for k in range(2)]
            wiB = [cpool.tile([128, 128], F16, name=f"wiB{k}") for k in range(2)]
            for k in range(2):
                nc.sync.dma_start(out=wiA[k][:], in_=d_wiA[k * 128:(k + 1) * 128, :])
                nc.sync.dma_start(out=wiB[k][:], in_=d_wiB[k * 128:(k + 1) * 128, :])
            biasA = cpool.tile([128, 1], F32, name="biasA")
            nc.sync.dma_start(out=biasA[:], in_=d_biasA[:])
            biasB = cpool.tile([128, 1], F32, name="biasB")
            nc.sync.dma_start(out=biasB[:], in_=d_biasB[:])

            ident16 = cpool.tile([128, 128], F16)
            make_identity(nc, ident16[:])

            wo16 = []
            for k, d_wo in enumerate((d_wo0, d_wo1)):
                wo = cpool.tile([128, VS], F16, name=f"wo16_{k}")
                nc.sync.dma_start(out=wo[:], in_=d_wo[:])
                wo16.append(wo)

            # fp16 history tiles, one per token tile; col = b*8 + c
            ht0 = [hpool.tile([128, 128], F16, tag="ht0", bufs=NT,
                              name=f"ht0_{g}") for g in range(NT)]
            ht1 = [hpool.tile([128, 128], F16, tag="ht1", bufs=NT,
                              name=f"ht1_{g}") for g in range(NT)]
            # tile 31's pad column (c=7) is read by the projection
            nc.vector.memset(ht0[NT - 1][:], 0.0)
            nc.vector.memset(ht1[NT - 1][:], 0.0)

            def hv(ht_g, r0, r1, c):
                # [r1-r0, 16] column view of step slot c (stride 8, offset c)
                return ht_g[r0:r1].rearrange("p (b t) -> p b t", t=8)[:, :, c]

            # ---------------- phase A: gather -> embT -> U in PSUM ----------
            bankA = {}
            bankB = {}
            gth_tiles = {}
            embt_tiles = {}

            def issue_gather(g):
                gth = wpool.tile([128, E], F32, tag="gather", bufs=6,
                                 name=f"gth_{g}")
                nc.gpsimd.indirect_dma_start(
                    out=gth[:], out_offset=None, in_=d_emb[:],
                    in_offset=bass.IndirectOffsetOnAxis(ap=ids_sb[:, g:g + 1], axis=0),
                )
                gth_tiles[g] = gth

            g16_tiles = {}

            def cast_emb(g):
                # fp32 -> fp16 on gpsimd (~1us); issued well before the PE
                # transposes that read it
                gth = gth_tiles.pop(g)
                g16 = wpool.tile([128, E], F16, tag="g16", bufs=2, name=f"g16_{g}")
                nc.gpsimd.tensor_copy(g16[:], gth[:])
                g16_tiles[g] = g16

            def prep_embt(g):
                # transpose on the PE, copy out on DVE
                g16 = g16_tiles.pop(g)
                embt = []
                for k in range(2):
                    tp = psum.tile([128, 128], F16, tag="tp", bufs=2, space="PSUM",
                                   name=f"tp_{g}_{k}")
                    nc.tensor.transpose(
                        out=tp[:], in_=g16[:, k * 128:(k + 1) * 128],
                        identity=ident16[:])
                    et = wpool.tile([128, 128], F16, tag=f"embt{k}", bufs=3,
                                    name=f"et_{g}_{k}")
                    nc.vector.tensor_copy(et[:], tp[:])
                    embt.append(et)
                embt_tiles[g] = embt

            def u_mms(g):
                embt = embt_tiles.pop(g)
                # One shared PSUM bank: cols 0:128 = blocks 0,1 (all 8 step
                # slots), cols 128:160 = blocks 3,2 at step slots c=0 / c=4.
                # The FIRST matmul's start=True marks the whole 2KB zero
                # region pending-zero, so every other matmul accumulates
                # with start=False.
                ab = psum.tile([128, 160], F32, tag="bankAB", bufs=2,
                               space="PSUM", name=f"bankAB_{g}")
                for k in range(2):
                    src = embt[k][:].rearrange("p (c2 r) -> p c2 r", c2=2)
                    nc.tensor.matmul(out=ab[:, 128:160], lhsT=wiB[k][:],
                                     rhs=src[:, :, 0:16],
                                     start=(k == 0), stop=(k == 1))
                for k in range(2):
                    nc.tensor.matmul(out=ab[:, 0:128], lhsT=wiA[k][:],
                                     rhs=embt[k][:],
                                     start=False, stop=True,
                                     skip_group_check=True)
                bankA[g] = ab
                bankB[g] = ab

            for g0 in range(4):
                issue_gather(g0)
            cast_emb(0)
            cast_emb(1)
            prep_embt(0)
            cast_emb(2)
            prep_embt(1)
            u_mms(0)

            # ---------------- projection pacing ----------------
            from collections import deque
            proj_q = deque()   # pending (g, unit) items; unit = (p, k, vc)
            ob_tiles = {}
            done_chunks = {}

            def enqueue_proj(g):
                # 8 units of 2 matmuls each; k0 units start a pair of PSUM
                # banks, k1 units finish + drain them
                for p in range(2):
                    for pair in range(2):
                        for k in range(2):
                            proj_q.append((g, p, pair, k))

            pp_banks = {}

            def emit_proj_unit():
                if not proj_q:
                    return
                g, p, pair, k = proj_q.popleft()
                if g not in ob_tiles:
                    ob_tiles[g] = opool.tile([128, VS], F16, tag="ob",
                                             name=f"ob_{g}")
                    done_chunks[g] = 0
                ht_g = ht0[g] if k == 0 else ht1[g]
                drains = []
                for vc in (2 * pair, 2 * pair + 1):
                    col = p * 2000 + vc * VC
                    if k == 0:
                        pp = psum.tile([128, VC], F32, tag="pp", bufs=4,
                                       space="PSUM", name=f"pp_{g}_{p}_{vc}")
                        pp_banks[(g, p, vc)] = pp
                        nc.tensor.matmul(out=pp[:], lhsT=ht_g[:],
                                         rhs=wo16[0][:, col:col + VC],
                                         start=True, stop=False)
                    else:
                        pp = pp_banks.pop((g, p, vc))
                        nc.tensor.matmul(out=pp[:], lhsT=ht_g[:],
                                         rhs=wo16[1][:, col:col + VC],
                                         start=False, stop=True)
                        drains.append((col, pp))
                for col, pp in drains:
                    # all drains on DVE: ACT must stay clear for the chain
                    # tanhs, gpsimd has no PSUM access
                    nc.vector.tensor_copy(ob_tiles[g][:, col:col + VC], pp[:])
                done_chunks[g] += len(drains)
                if done_chunks[g] == 8:
                    ob = ob_tiles.pop(g)
                    nc.sync.dma_start(out=d_out[g * 128:(g + 1) * 128, :],
                                      in_=ob[:])

            # ---------------- serial chains ----------------
            # per-step emission; chain1/2/3 are slotted to lag chain0.
            for t in range(T):
                g, c = divmod(t, 8)

                # --- projection first: keeps PE busy while chain waits ---
                if c == 0 and g >= 1:
                    enqueue_proj(g - 1)
                emit_proj_unit()

                if c == 0 and g + 4 < NT:
                    issue_gather(g + 4)

                # --- chain0 (block0, every step) ---
                dst0 = hv(ht0[g], 0, 64, c)
                if t == 0:
                    nc.scalar.activation(dst0, bankA[0][0:64, 0:16], TANH,
                                         bias=biasA[0:64])
                else:
                    src = hv(ht0[g - 1], 0, 64, 7) if c == 0 else \
                        hv(ht0[g], 0, 64, c - 1)
                    nc.tensor.matmul(out=bankA[g][0:64, c * 16:(c + 1) * 16],
                                     lhsT=whA[0:64, 0:64], rhs=src,
                                     start=False, stop=True,
                                     skip_group_check=True)
                    nc.scalar.activation(dst0, bankA[g][0:64, c * 16:(c + 1) * 16],
                                         TANH, bias=biasA[0:64])

                # --- chain1 (block1, even steps) ---
                if c % 2 == 0:
                    dst1 = hv(ht0[g], 64, 128, c)
                    cc = slice(c * 16, (c + 1) * 16)
                    if t == 0:
                        nc.scalar.activation(dst1, bankA[0][64:128, 0:16], TANH,
                                             bias=biasA[64:128])
                    else:
                        self_src = hv(ht0[g], 64, 128, c - 2) if c >= 2 else \
                            hv(ht0[g - 1], 64, 128, 6)
                        nc.tensor.matmul(out=bankA[g][64:128, cc],
                                         lhsT=whA[64:128, 64:128], rhs=self_src,
                                         start=False, stop=True,
                                         skip_group_check=True)
                        nc.scalar.activation(dst1, bankA[g][64:128, cc], TANH,
                                             bias=biasA[64:128])
                    # held value for the odd step c+1 (off critical path)
                    v1 = ht0[g][64:128].rearrange("p (b t) -> p b t", t=8)
                    nc.gpsimd.tensor_copy(v1[:, :, c + 1], dst1)

                # --- cross block0 -> block1 for col c+1 (odd c) ---
                if c in (1, 3, 5) and t + 1 < T:
                    cc1 = slice((c + 1) * 16, (c + 2) * 16)
                    nc.tensor.matmul(out=bankA[g][64:128, cc1],
                                     lhsT=whA[0:64, 64:128],
                                     rhs=hv(ht0[g], 0, 64, c),
                                     start=False, stop=True,
                                     skip_group_check=True)
                if c == 7 and g + 1 < NT:
                    # cross into next tile's col 0
                    nc.tensor.matmul(out=bankA[g + 1][64:128, 0:16],
                                     lhsT=whA[0:64, 64:128],
                                     rhs=hv(ht0[g], 0, 64, 7),
                                     start=False, stop=True,
                                     skip_group_check=True)

                # --- chain3 (block3, t%8==0), slotted at c==1 ---
                if c == 1:
                    dst3 = hv(ht1[g], 0, 64, 0)
                    if g == 0:
                        nc.scalar.activation(dst3, bankB[0][0:64, 128:144], TANH,
                                             bias=biasB[0:64])
                    else:
                        # cross from blocks 0,1 at t-1
                        nc.tensor.matmul(out=bankB[g][0:64, 128:144],
                                         lhsT=wX3[:],
                                         rhs=ht0[g - 1][:].rearrange(
                                             "p (b t) -> p b t", t=8)[:, :, 7],
                                         start=False, stop=True,
                                         skip_group_check=True)
                        # self W33 + W23 (block2 state held at col 4)
                        nc.tensor.matmul(out=bankB[g][0:64, 128:144],
                                         lhsT=wB3[:],
                                         rhs=ht1[g - 1][:].rearrange(
                                             "p (b t) -> p b t", t=8)[:, :, 4],
                                         start=False, stop=True,
                                         skip_group_check=True)
                        nc.scalar.activation(dst3, bankB[g][0:64, 128:144], TANH,
                                             bias=biasB[0:64])
                    v3 = ht1[g][0:64].rearrange("p (b t) -> p b t", t=8)
                    nc.gpsimd.tensor_copy(
                        v3[:, :, 1:8],
                        dst3[:, :, None].to_broadcast([64, B, 7]))

                if c == 1 and g + 2 < NT:
                    prep_embt(g + 2)
                if c == 6 and g + 3 < NT:
                    cast_emb(g + 3)
                if c == 5 and g + 1 < NT:
                    u_mms(g + 1)

                # --- chain2 (block2, t%4==0), slotted at c==2 and c==5 ---
                if c == 2 or c == 5:
                    cs = 0 if c == 2 else 4          # step slot being computed
                    bb_cols = slice(128, 144) if cs == 0 else slice(144, 160)
                    dst2 = hv(ht1[g], 64, 128, cs)
                    if t <= 2:
                        nc.scalar.activation(dst2, bankB[0][64:128, bb_cols],
                                             TANH, bias=biasB[64:128])
                    else:
                        # cross from blocks 0,1 at t-1
                        xsrc = ht0[g - 1][:].rearrange(
                            "p (b t) -> p b t", t=8)[:, :, 7] if cs == 0 else \
                            ht0[g][:].rearrange(
                                "p (b t) -> p b t", t=8)[:, :, 3]
                        nc.tensor.matmul(out=bankB[g][64:128, bb_cols],
                                         lhsT=wX2[:], rhs=xsrc,
                                         start=False, stop=True,
                                         skip_group_check=True)
                        self_src = hv(ht1[g - 1], 64, 128, 4) if cs == 0 else \
                            hv(ht1[g], 64, 128, 0)
                        nc.tensor.matmul(out=bankB[g][64:128, bb_cols],
                                         lhsT=wB2[64:128, :], rhs=self_src,
                                         start=False, stop=True,
                                         skip_group_check=True)
                        nc.scalar.activation(dst2, bankB[g][64:128, bb_cols],
                                             TANH, bias=biasB[64:128])
                    span = 3 if cs == 0 else min(3, T - t + 1)
                    v2 = ht1[g][64:128].rearrange("p (b t) -> p b t", t=8)
                    nc.gpsimd.tensor_copy(
                        v2[:, :, cs + 1:cs + 1 + span],
                        dst2[:, :, None].to_broadcast([64, B, span]))

            # flush remaining projection work (tile 30 leftovers + tile 31)
            enqueue_proj(NT - 1)
            while proj_q:
                emit_proj_unit()

            if DEBUG_HT:
                for g in range(NT):
                    # dbg row = g*128 + unit_partition, col = token slot b*8+c
                    nc.sync.dma_start(out=d_dbg[g * 128:(g + 1) * 128, 0:128],
                                      in_=ht0[g][:])
                    nc.sync.dma_start(out=d_dbg[g * 128:(g + 1) * 128, 128:256],
                                      in_=ht1[g][:])

    nc.finalize()
    return nc


_NC_CACHE = None
TRACE = False        # set by test harness to capture an NTFF profile
TRACE_KW = {}
LAST_RESULT = None   # BassKernelResults of the most recent run
DEBUG_HT = False     # add a debug output with the recorded h history


def kernel(x, x_sl, embedding, Wi, Wh, bi, bh, Wo):
    global _NC_CACHE, LAST_RESULT
    if _NC_CACHE is None:
        _NC_CACHE = build_program()
    nc = _NC_CACHE

    x = np.asarray(x)
    ids = np.ascontiguousarray(x[:, :T].T).reshape(-1)  # n = t*B + b
    ids_pad = np.zeros(128 * NT, np.int32)
    ids_pad[:B * T] = ids
    ids_dev = np.ascontiguousarray(ids_pad.reshape(NT, 128).T)

    embedding = np.ascontiguousarray(np.asarray(embedding, np.float32))
    Wh16 = np.asarray(Wh, np.float16)
    Wi16 = np.asarray(Wi, np.float16)
    biasv = (np.asarray(bi, np.float32) + np.asarray(bh, np.float32))
    Wo16 = np.asarray(Wo, np.float16)

    whA_h = np.ascontiguousarray(Wh16[0:128, 0:128])
    wB2_h = np.zeros((128, 64), np.float16)
    wB2_h[64:128] = Wh16[128:192, 128:192]
    wB3_h = np.zeros((128, 64), np.float16)
    wB3_h[0:64] = Wh16[192:256, 192:256]
    wB3_h[64:128] = Wh16[128:192, 192:256]
    wX2_h = np.ascontiguousarray(Wh16[0:128, 128:192])
    wX3_h = np.ascontiguousarray(Wh16[0:128, 192:256])
    wiA_h = np.ascontiguousarray(Wi16[:, 0:128])
    wiB_h = np.ascontiguousarray(
        np.concatenate([Wi16[:, 192:256], Wi16[:, 128:192]], axis=1))
    biasA_h = np.ascontiguousarray(biasv[0:128].reshape(128, 1))
    biasB_h = np.ascontiguousarray(
        np.concatenate([biasv[192:256], biasv[128:192]]).reshape(128, 1))

    in_maps = []
    for cidx in range(NCORES):
        sl = slice(cidx * VS, (cidx + 1) * VS)
        in_maps.append({
            "tok_ids": ids_dev,
            "embedding": embedding,
            "whA": whA_h, "wB2": wB2_h, "wB3": wB3_h,
            "wX2": wX2_h, "wX3": wX3_h,
            "wiA": wiA_h, "wiB": wiB_h,
            "biasA": biasA_h, "biasB": biasB_h,
            "wo0": np.ascontiguousarray(Wo16[0:128, sl]),
            "wo1": np.ascontiguousarray(
                np.concatenate([Wo16[192:256, sl], Wo16[128:192, sl]], axis=0)),
        })

    res = run_bass_kernel_spmd(nc, in_maps, core_ids=list(range(NCORES)),
                               trace=TRACE, **TRACE_KW)
    LAST_RESULT = res
    raw = np.concatenate([r["out"] for r in res.results], axis=1)  # [4096, V]
    out = raw.reshape(NT, B, 8, V).transpose(1, 0, 2, 3).reshape(B, NT * 8, V)
    return out[:, :T].astype(np.float32)


# revision 18
# speedup vs baseline: 1.0040x; 1.0040x over previous
"""CWRNN language-model kernel for 8 Trainium2 NeuronCores.

Strategy (vocab-sharded output projection, v2):
  - Each core owns Wo[:, c*4000:(c+1)*4000] and writes its logits slice in
    fp16 (tolerance 2e-2 >> fp16 rounding) -> halves the HBM write volume
    that dominated v1.
  - The clockwork mask is block-triangular: block0 (period 1, units 0:64)
    is fed only by itself, so the serial critical path is a 64-unit RNN.
    Blocks 1-3 run as separate serial chains lagging block0, with their
    cross-block input terms batched as per-tile matmuls over the recorded
    history tiles.
  - U = emb @ Wi is accumulated directly into PSUM banks (phase A); chain
    matmuls accumulate h @ Whh on top (start=False) and tanh reads PSUM
    with the bias folded into the activation -> each chain step is exactly
    matmul -> tanh, and tanh writes straight into the fp16 history tile
    column the next step's matmul reads (no DVE add, no Pool copy on the
    critical path).
  - Projection pairs two 500-col chunks per weight load, paced one unit
    per chain step two tiles behind the recurrence; all PSUM drains stay
    on DVE (ACT is reserved for the serial tanh chain; gpsimd has no PSUM
    access); one fp16 output DMA per tile on the sync queue.
  - Pacing note: the PE sustains only ~50% of its 2.4GHz peak (power
    throttle).  Denser projection pacing (more PSUM banks, bigger bursts)
    measurably LOWERS throughput once the throttle engages, so the
    drain-coupled pp=2 pacing is deliberate.
"""

import sys

sys.path.insert(0, "/opt/trn_rl_repo")

import numpy as np

import concourse.bass as bass
import concourse.mybir as mybir
import concourse.tile as tile
from concourse import bacc
from concourse import bass_utils as _bass_utils
from concourse.bass_utils import run_bass_kernel_spmd
from concourse.masks import make_identity

# note: walrus's --enable-ldw-opt pass crashes codegen on this toolchain
# (visitInstLdweights), so per-matmul LDWEIGHTS reloads are unavoidable

F32 = mybir.dt.float32
F16 = mybir.dt.float16
I32 = mybir.dt.int32
TANH = mybir.ActivationFunctionType.Tanh

B = 16
T = 255           # x[:, :-1]
E = 256
NH = 256
V = 32000
NCORES = 8
VS = V // NCORES  # 4000 vocab columns per core
NT = 32           # token tiles of 8 steps (tile 31 has 7 real steps)
VC = 500          # vocab chunk per PSUM bank
PROJ_LAG = 2      # tiles between recurrence and projection start


def build_program():
    nc = bacc.Bacc(target_bir_lowering=False)

    d_ids = nc.dram_tensor("tok_ids", [128, NT], I32, kind="ExternalInput")
    d_emb = nc.dram_tensor("embedding", [V + 1, E], F32, kind="ExternalInput")
    d_whA = nc.dram_tensor("whA", [128, 128], F16, kind="ExternalInput")
    d_wB2 = nc.dram_tensor("wB2", [128, 64], F16, kind="ExternalInput")
    d_wB3 = nc.dram_tensor("wB3", [128, 64], F16, kind="ExternalInput")
    d_wX2 = nc.dram_tensor("wX2", [128, 64], F16, kind="ExternalInput")
    d_wX3 = nc.dram_tensor("wX3", [128, 64], F16, kind="ExternalInput")
    d_wiA = nc.dram_tensor("wiA", [256, 128], F16, kind="ExternalInput")
    d_wiB = nc.dram_tensor("wiB", [256, 128], F16, kind="ExternalInput")
    d_biasA = nc.dram_tensor("biasA", [128, 1], F32, kind="ExternalInput")
    d_biasB = nc.dram_tensor("biasB", [128, 1], F32, kind="ExternalInput")
    d_wo0 = nc.dram_tensor("wo0", [128, VS], F16, kind="ExternalInput")
    d_wo1 = nc.dram_tensor("wo1", [128, VS], F16, kind="ExternalInput")
    # raw token-major output: row g*128 + b*8 + c  <->  logits[b, g*8+c]
    d_out = nc.dram_tensor("out", [NT * 128, VS], F16, kind="ExternalOutput")
    d_dbg = None
    if DEBUG_HT:
        d_dbg = nc.dram_tensor("dbg_ht", [NT * 128, 256], F16,
                               kind="ExternalOutput")

    with tile.TileContext(nc) as tc:
        with tc.tile_pool(name="const", bufs=1) as cpool, \
             tc.tile_pool(name="hist", bufs=1) as hpool, \
             tc.tile_pool(name="obuf", bufs=3) as opool, \
             tc.tile_pool(name="work", bufs=3) as wpool, \
             tc.tile_pool(name="psum", bufs=2, space="PSUM") as psum:

            # ---------------- constants and weights ----------------
            ids_sb = cpool.tile([128, NT], I32)
            nc.sync.dma_start(out=ids_sb[:], in_=d_ids[:])

            whA = cpool.tile([128, 128], F16, name="whA")
            nc.sync.dma_start(out=whA[:], in_=d_whA[:])
            wB2 = cpool.tile([128, 64], F16, name="wB2")
            nc.sync.dma_start(out=wB2[:], in_=d_wB2[:])
            wB3 = cpool.tile([128, 64], F16, name="wB3")
            nc.sync.dma_start(out=wB3[:], in_=d_wB3[:])
            wX2 = cpool.tile([128, 64], F16, name="wX2")
            nc.sync.dma_start(out=wX2[:], in_=d_wX2[:])
            wX3 = cpool.tile([128, 64], F16, name="wX3")
            nc.sync.dma_start(out=wX3[:], in_=d_wX3[:])
            wiA = [cpool.tile([128, 128], F16, name=f"wiA{k}") for k in range(2)]
            wiB = [cpool.tile([128, 128], F16, name=f"wiB{k}") for k in range(2)]
            for k in range(2):
                nc.sync.dma_start(out=wiA[k][:], in_=d_wiA[k * 128:(k + 1) * 128, :])
                nc.sync.dma_start(out=wiB[k][:], in_=d_wiB[k * 128:(k + 1) * 128, :])
            biasA = cpool.tile([128, 1], F32, name="biasA")
            nc.sync.dma_start(out=biasA[:], in_=d_biasA[:])
            biasB = cpool.tile([128, 1], F32, name="biasB")
            nc.sync.dma_start(out=biasB[:], in_=d_biasB[:])

            ident16 = cpool.tile([128, 128], F16)
            make_identity(nc, ident16[:])

            wo16 = []
            for k, d_wo in enumerate((d_wo0, d_wo1)):
                wo = cpool.tile([128, VS], F16, name=f"wo16_{k}")
                nc.sync.dma_start(out=wo[:], in_=d_wo[:])
                wo16.append(wo)

            # fp16 history tiles, one per token tile; col = b*8 + c
            ht0 = [hpool.tile([128, 128], F16, tag="ht0", bufs=NT,
                              name=f"ht0_{g}") for g in range(NT)]
            ht1 = [hpool.tile([128, 128], F16, tag="ht1", bufs=NT,
                              name=f"ht1_{g}") for g in range(NT)]
            # tile 31's pad column (c=7) is read by the projection
            nc.vector.memset(ht0[NT - 1][:], 0.0)
            nc.vector.memset(ht1[NT - 1][:], 0.0)

            def hv(ht_g, r0, r1, c):
                # [r1-r0, 16] column view of step slot c (stride 8, offset c)
                return ht_g[r0:r1].rearrange("p (b t) -> p b t", t=8)[:, :, c]

            # ---------------- phase A: gather -> embT -> U in PSUM ----------
            bankA = {}
            bankB = {}
            gth_tiles = {}
            embt_tiles = {}

            def issue_gather(g):
                gth = wpool.tile([128, E], F32, tag="gather", bufs=6,
                                 name=f"gth_{g}")
                nc.gpsimd.indirect_dma_start(
                    out=gth[:], out_offset=None, in_=d_emb[:],
                    in_offset=bass.IndirectOffsetOnAxis(ap=ids_sb[:, g:g + 1], axis=0),
                )
                gth_tiles[g] = gth

            g16_tiles = {}

            def cast_emb(g):
                # fp32 -> fp16 on gpsimd (~1us); issued well before the PE
                # transposes that read it
                gth = gth_tiles.pop(g)
                g16 = wpool.tile([128, E], F16, tag="g16", bufs=2, name=f"g16_{g}")
                nc.gpsimd.tensor_copy(g16[:], gth[:])
                g16_tiles[g] = g16

            def prep_embt(g):
                # transpose on the PE, copy out on DVE
                g16 = g16_tiles.pop(g)
                embt = []
                for k in range(2):
                    tp = psum.tile([128, 128], F16, tag="tp", bufs=2, space="PSUM",
                                   name=f"tp_{g}_{k}")
                    nc.tensor.transpose(
                        out=tp[:], in_=g16[:, k * 128:(k + 1) * 128],
                        identity=ident16[:])
                    et = wpool.tile([128, 128], F16, tag=f"embt{k}", bufs=3,
                                    name=f"et_{g}_{k}")
                    nc.vector.tensor_copy(et[:], tp[:])
                    embt.append(et)
                embt_tiles[g] = embt

            def u_mms(g):
                embt = embt_tiles.pop(g)
                # One shared PSUM bank: cols 0:128 = blocks 0,1 (all 8 step
                # slots), cols 128:160 = blocks 3,2 at step slots c=0 / c=4.
                # The FIRST matmul's start=True marks the whole 2KB zero
                # region pending-zero, so every other matmul accumulates
                # with start=False.
                ab = psum.tile([128, 160], F32, tag="bankAB", bufs=2,
                               space="PSUM", name=f"bankAB_{g}")
                for k in range(2):
                    src = embt[k][:].rearrange("p (c2 r) -> p c2 r", c2=2)
                    nc.tensor.matmul(out=ab[:, 128:160], lhsT=wiB[k][:],
                                     rhs=src[:, :, 0:16],
                                     start=(k == 0), stop=(k == 1))
                for k in range(2):
                    nc.tensor.matmul(out=ab[:, 0:128], lhsT=wiA[k][:],
                                     rhs=embt[k][:],
                                     start=False, stop=True,
                                     skip_group_check=True)
                bankA[g] = ab
                bankB[g] = ab

            for g0 in range(4):
                issue_gather(g0)
            cast_emb(0)
            cast_emb(1)
            prep_embt(0)
            cast_emb(2)
            prep_embt(1)
            u_mms(0)

            # ---------------- projection pacing ----------------
            from collections import deque
            proj_q = deque()   # pending (g, unit) items; unit = (p, k, vc)
            ob_tiles = {}
            done_chunks = {}

            def enqueue_proj(g):
                # 8 units of 2 matmuls each; k0 units start a pair of PSUM
                # banks, k1 units finish + drain them
                for p in range(2):
                    for pair in range(2):
                        for k in range(2):
                            proj_q.append((g, p, pair, k))

            pp_banks = {}

            def emit_proj_unit():
                if not proj_q:
                    return
                g, p, pair, k = proj_q.popleft()
                if g not in ob_tiles:
                    ob_tiles[g] = opool.tile([128, VS], F16, tag="ob",
                                             name=f"ob_{g}")
                    done_chunks[g] = 0
                ht_g = ht0[g] if k == 0 else ht1[g]
                drains = []
                for vc in (2 * pair, 2 * pair + 1):
                    col = p * 2000 + vc * VC
                    if k == 0:
                        pp = psum.tile([128, VC], F32, tag="pp", bufs=4,
                                       space="PSUM", name=f"pp_{g}_{p}_{vc}")
                        pp_banks[(g, p, vc)] = pp
                        nc.tensor.matmul(out=pp[:], lhsT=ht_g[:],
                                         rhs=wo16[0][:, col:col + VC],
                                         start=True, stop=False)
                    else:
                        pp = pp_banks.pop((g, p, vc))
                        nc.tensor.matmul(out=pp[:], lhsT=ht_g[:],
                                         rhs=wo16[1][:, col:col + VC],
                                         start=False, stop=True)
                        drains.append((col, pp))
                for col, pp in drains:
                    # all drains on DVE: ACT must stay clear for the chain
                    # tanhs, gpsimd has no PSUM access
                    nc.vector.tensor_copy(ob_tiles[g][:, col:col + VC], pp[:])
                done_chunks[g] += len(drains)
                if done_chunks[g] == 8:
                    ob = ob_tiles.pop(g)
                    nc.sync.dma_start(out=d_out[g * 128:(g + 1) * 128, :],
                                      in_=ob[:])

            # ---------------- serial chains ----------------
            # per-step emission; chain1/2/3 are slotted to lag chain0.
            for t in range(T):
                g, c = divmod(t, 8)

                if c == 0 and g + 4 < NT:
                    issue_gather(g + 4)

                # --- chain0 (block0, every step) ---
                dst0 = hv(ht0[g], 0, 64, c)
                if t == 0:
                    nc.scalar.activation(dst0, bankA[0][0:64, 0:16], TANH,
                                         bias=biasA[0:64])
                else:
                    src = hv(ht0[g - 1], 0, 64, 7) if c == 0 else \
                        hv(ht0[g], 0, 64, c - 1)
                    nc.tensor.matmul(out=bankA[g][0:64, c * 16:(c + 1) * 16],
                                     lhsT=whA[0:64, 0:64], rhs=src,
                                     start=False, stop=True,
                                     skip_group_check=True)
                    nc.scalar.activation(dst0, bankA[g][0:64, c * 16:(c + 1) * 16],
                                         TANH, bias=biasA[0:64])

                # --- chain1 (block1, even steps) ---
                if c % 2 == 0:
                    dst1 = hv(ht0[g], 64, 128, c)
                    cc = slice(c * 16, (c + 1) * 16)
                    if t == 0:
                        nc.scalar.activation(dst1, bankA[0][64:128, 0:16], TANH,
                                             bias=biasA[64:128])
                    else:
                        self_src = hv(ht0[g], 64, 128, c - 2) if c >= 2 else \
                            hv(ht0[g - 1], 64, 128, 6)
                        nc.tensor.matmul(out=bankA[g][64:128, cc],
                                         lhsT=whA[64:128, 64:128], rhs=self_src,
                                         start=False, stop=True,
                                         skip_group_check=True)
                        nc.scalar.activation(dst1, bankA[g][64:128, cc], TANH,
                                             bias=biasA[64:128])
                    # held value for the odd step c+1 (off critical path)
                    v1 = ht0[g][64:128].rearrange("p (b t) -> p b t", t=8)
                    nc.gpsimd.tensor_copy(v1[:, :, c + 1], dst1)

                # --- cross block0 -> block1 for col c+1 (odd c) ---
                if c in (1, 3, 5) and t + 1 < T:
                    cc1 = slice((c + 1) * 16, (c + 2) * 16)
                    nc.tensor.matmul(out=bankA[g][64:128, cc1],
                                     lhsT=whA[0:64, 64:128],
                                     rhs=hv(ht0[g], 0, 64, c),
                                     start=False, stop=True,
                                     skip_group_check=True)
                if c == 7 and g + 1 < NT:
                    # cross into next tile's col 0
                    nc.tensor.matmul(out=bankA[g + 1][64:128, 0:16],
                                     lhsT=whA[0:64, 64:128],
                                     rhs=hv(ht0[g], 0, 64, 7),
                                     start=False, stop=True,
                                     skip_group_check=True)

                # --- chain3 (block3, t%8==0), slotted at c==1 ---
                if c == 1:
                    dst3 = hv(ht1[g], 0, 64, 0)
                    if g == 0:
                        nc.scalar.activation(dst3, bankB[0][0:64, 128:144], TANH,
                                             bias=biasB[0:64])
                    else:
                        # cross from blocks 0,1 at t-1
                        nc.tensor.matmul(out=bankB[g][0:64, 128:144],
                                         lhsT=wX3[:],
                                         rhs=ht0[g - 1][:].rearrange(
                                             "p (b t) -> p b t", t=8)[:, :, 7],
                                         start=False, stop=True,
                                         skip_group_check=True)
                        # self W33 + W23 (block2 state held at col 4)
                        nc.tensor.matmul(out=bankB[g][0:64, 128:144],
                                         lhsT=wB3[:],
                                         rhs=ht1[g - 1][:].rearrange(
                                             "p (b t) -> p b t", t=8)[:, :, 4],
                                         start=False, stop=True,
                                         skip_group_check=True)
                        nc.scalar.activation(dst3, bankB[g][0:64, 128:144], TANH,
                                             bias=biasB[0:64])
                    v3 = ht1[g][0:64].rearrange("p (b t) -> p b t", t=8)
                    nc.gpsimd.tensor_copy(
                        v3[:, :, 1:8],
                        dst3[:, :, None].to_broadcast([64, B, 7]))

                if c == 1 and g + 2 < NT:
                    prep_embt(g + 2)
                if c == 6 and g + 3 < NT:
                    cast_emb(g + 3)
                if c == 5 and g + 1 < NT:
                    u_mms(g + 1)

                # --- chain2 (block2, t%4==0), slotted at c==2 and c==5 ---
                if c == 2 or c == 5:
                    cs = 0 if c == 2 else 4          # step slot being computed
                    bb_cols = slice(128, 144) if cs == 0 else slice(144, 160)
                    dst2 = hv(ht1[g], 64, 128, cs)
                    if t <= 2:
                        nc.scalar.activation(dst2, bankB[0][64:128, bb_cols],
                                             TANH, bias=biasB[64:128])
                    else:
                        # cross from blocks 0,1 at t-1
                        xsrc = ht0[g - 1][:].rearrange(
                            "p (b t) -> p b t", t=8)[:, :, 7] if cs == 0 else \
                            ht0[g][:].rearrange(
                                "p (b t) -> p b t", t=8)[:, :, 3]
                        nc.tensor.matmul(out=bankB[g][64:128, bb_cols],
                                         lhsT=wX2[:], rhs=xsrc,
                                         start=False, stop=True,
                                         skip_group_check=True)
                        self_src = hv(ht1[g - 1], 64, 128, 4) if cs == 0 else \
                            hv(ht1[g], 64, 128, 0)
                        nc.tensor.matmul(out=bankB[g][64:128, bb_cols],
                                         lhsT=wB2[64:128, :], rhs=self_src,
                                         start=False, stop=True,
                                         skip_group_check=True)
                        nc.scalar.activation(dst2, bankB[g][64:128, bb_cols],
                                             TANH, bias=biasB[64:128])
                    span = 3 if cs == 0 else min(3, T - t + 1)
                    v2 = ht1[g][64:128].rearrange("p (b t) -> p b t", t=8)
                    nc.gpsimd.tensor_copy(
                        v2[:, :, cs + 1:cs + 1 + span],
                        dst2[:, :, None].to_broadcast([64, B, span]))

                # --- projection pacing: 1 unit (2 matmuls) per step ---
                if c == 7 and g >= PROJ_LAG:
                    enqueue_proj(g - PROJ_LAG)
                emit_proj_unit()

            # flush remaining projection work (tiles whose c==7 enqueue
            # never fired: the last PROJ_LAG tiles plus tile NT-1 itself)
            for g in range(NT - PROJ_LAG - 1, NT):
                enqueue_proj(g)
            while proj_q:
                emit_proj_unit()

            if DEBUG_HT:
                for g in range(NT):
                    # dbg row = g*128 + unit_partition, col = token slot b*8+c
                    nc.sync.dma_start(out=d_dbg[g * 128:(g + 1) * 128, 0:128],
                                      in_=ht0[g][:])
                    nc.sync.dma_start(out=d_dbg[g * 128:(g + 1) * 128, 128:256],
                                      in_=ht1[g][:])

    nc.finalize()
    return nc


_NC_CACHE = None
TRACE = False        # set by test harness to capture an NTFF profile
TRACE_KW = {}
LAST_RESULT = None   # BassKernelResults of the most recent run
DEBUG_HT = False     # add a debug output with the recorded h history


def kernel(x, x_sl, embedding, Wi, Wh, bi, bh, Wo):
    global _NC_CACHE, LAST_RESULT
    if _NC_CACHE is None:
        _NC_CACHE = build_program()
    nc = _NC_CACHE

    x = np.asarray(x)
    ids = np.ascontiguousarray(x[:, :T].T).reshape(-1)  # n = t*B + b
    ids_pad = np.zeros(128 * NT, np.int32)
    ids_pad[:B * T] = ids
    ids_dev = np.ascontiguousarray(ids_pad.reshape(NT, 128).T)

    embedding = np.ascontiguousarray(np.asarray(embedding, np.float32))
    Wh16 = np.asarray(Wh, np.float16)
    Wi16 = np.asarray(Wi, np.float16)
    biasv = (np.asarray(bi, np.float32) + np.asarray(bh, np.float32))
    Wo16 = np.asarray(Wo, np.float16)

    whA_h = np.ascontiguousarray(Wh16[0:128, 0:128])
    wB2_h = np.zeros((128, 64), np.float16)
    wB2_h[64:128] = Wh16[128:192, 128:192]
    wB3_h = np.zeros((128, 64), np.float16)
    wB3_h[0:64] = Wh16[192:256, 192:256]
    wB3_h[64:128] = Wh16[128:192, 192:256]
    wX2_h = np.ascontiguousarray(Wh16[0:128, 128:192])
    wX3_h = np.ascontiguousarray(Wh16[0:128, 192:256])
    wiA_h = np.ascontiguousarray(Wi16[:, 0:128])
    wiB_h = np.ascontiguousarray(
        np.concatenate([Wi16[:, 192:256], Wi16[:, 128:192]], axis=1))
    biasA_h = np.ascontiguousarray(biasv[0:128].reshape(128, 1))
    biasB_h = np.ascontiguousarray(
        np.concatenate([biasv[192:256], biasv[128:192]]).reshape(128, 1))

    in_maps = []
    for cidx in range(NCORES):
        sl = slice(cidx * VS, (cidx + 1) * VS)
        in_maps.append({
            "tok_ids": ids_dev,
            "embedding": embedding,
            "whA": whA_h, "wB2": wB2_h, "wB3": wB3_h,
            "wX2": wX2_h, "wX3": wX3_h,
            "wiA": wiA_h, "wiB": wiB_h,
            "biasA": biasA_h, "biasB": biasB_h,
            "wo0": np.ascontiguousarray(Wo16[0:128, sl]),
            "wo1": np.ascontiguousarray(
                np.concatenate([Wo16[192:256, sl], Wo16[128:192, sl]], axis=0)),
        })

    res = run_bass_kernel_spmd(nc, in_maps, core_ids=list(range(NCORES)),
                               trace=TRACE, **TRACE_KW)
    LAST_RESULT = res
    raw = np.concatenate([r["out"] for r in res.results], axis=1)  # [4096, V]
    out = raw.reshape(NT, B, 8, V).transpose(1, 0, 2, 3).reshape(B, NT * 8, V)
    return out[:, :T].astype(np.float32)


# revision 19
# speedup vs baseline: 1.0355x; 1.0313x over previous
"""CWRNN language-model kernel for 8 Trainium2 NeuronCores.

Strategy (vocab-sharded output projection, v2):
  - Each core owns Wo[:, c*4000:(c+1)*4000] and writes its logits slice in
    fp16 (tolerance 2e-2 >> fp16 rounding) -> halves the HBM write volume
    that dominated v1.
  - The clockwork mask is block-triangular: block0 (period 1, units 0:64)
    is fed only by itself, so the serial critical path is a 64-unit RNN.
    Blocks 1-3 run as separate serial chains lagging block0, with their
    cross-block input terms batched as per-tile matmuls over the recorded
    history tiles.
  - U = emb @ Wi is accumulated directly into PSUM banks (phase A); chain
    matmuls accumulate h @ Whh on top (start=False) and tanh reads PSUM
    with the bias folded into the activation -> each chain step is exactly
    matmul -> tanh, and tanh writes straight into the fp16 history tile
    column the next step's matmul reads (no DVE add, no Pool copy on the
    critical path).
  - Projection pairs two 500-col chunks per weight load, paced one unit
    per chain step two tiles behind the recurrence; all PSUM drains stay
    on DVE (ACT is reserved for the serial tanh chain; gpsimd has no PSUM
    access); one fp16 output DMA per tile on the sync queue.
  - Pacing note: the PE sustains only ~50% of its 2.4GHz peak (power
    throttle).  Denser projection pacing (more PSUM banks, bigger bursts)
    measurably LOWERS throughput once the throttle engages, so the
    drain-coupled pp=2 pacing is deliberate.
"""

import sys

sys.path.insert(0, "/opt/trn_rl_repo")

import numpy as np

import concourse.bass as bass
import concourse.mybir as mybir
import concourse.tile as tile
from concourse import bacc
from concourse import bass_utils as _bass_utils
from concourse.bass_utils import run_bass_kernel_spmd
from concourse.masks import make_identity

# note: walrus's --enable-ldw-opt pass crashes codegen on this toolchain
# (visitInstLdweights), so per-matmul LDWEIGHTS reloads are unavoidable

F32 = mybir.dt.float32
F16 = mybir.dt.float16
I32 = mybir.dt.int32
TANH = mybir.ActivationFunctionType.Tanh

B = 16
T = 255           # x[:, :-1]
E = 256
NH = 256
V = 32000
NCORES = 8
VS = V // NCORES  # 4000 vocab columns per core
NT = 32           # token tiles of 8 steps (tile 31 has 7 real steps)
VC = 500          # vocab chunk per PSUM bank
PROJ_LAG = 2      # tiles between recurrence and projection start


def build_program():
    nc = bacc.Bacc(target_bir_lowering=False)

    d_ids = nc.dram_tensor("tok_ids", [128, NT], I32, kind="ExternalInput")
    d_emb = nc.dram_tensor("embedding", [V + 1, E], F32, kind="ExternalInput")
    d_whA = nc.dram_tensor("whA", [128, 128], F16, kind="ExternalInput")
    d_wB2 = nc.dram_tensor("wB2", [128, 64], F16, kind="ExternalInput")
    d_wB3 = nc.dram_tensor("wB3", [128, 64], F16, kind="ExternalInput")
    d_wX2 = nc.dram_tensor("wX2", [128, 64], F16, kind="ExternalInput")
    d_wX3 = nc.dram_tensor("wX3", [128, 64], F16, kind="ExternalInput")
    d_wiA = nc.dram_tensor("wiA", [256, 128], F16, kind="ExternalInput")
    d_wiB = nc.dram_tensor("wiB", [256, 128], F16, kind="ExternalInput")
    d_biasA = nc.dram_tensor("biasA", [128, 1], F32, kind="ExternalInput")
    d_biasB = nc.dram_tensor("biasB", [128, 1], F32, kind="ExternalInput")
    d_wo0 = nc.dram_tensor("wo0", [128, VS], F16, kind="ExternalInput")
    d_wo1 = nc.dram_tensor("wo1", [128, VS], F16, kind="ExternalInput")
    # raw token-major output: row g*128 + b*8 + c  <->  logits[b, g*8+c]
    d_out = nc.dram_tensor("out", [NT * 128, VS], F16, kind="ExternalOutput")
    d_dbg = None
    if DEBUG_HT:
        d_dbg = nc.dram_tensor("dbg_ht", [NT * 128, 256], F16,
                               kind="ExternalOutput")

    with tile.TileContext(nc) as tc:
        with tc.tile_pool(name="const", bufs=1) as cpool, \
             tc.tile_pool(name="hist", bufs=1) as hpool, \
             tc.tile_pool(name="obuf", bufs=3) as opool, \
             tc.tile_pool(name="work", bufs=3) as wpool, \
             tc.tile_pool(name="psum", bufs=2, space="PSUM") as psum:

            # ---------------- constants and weights ----------------
            ids_sb = cpool.tile([128, NT], I32)
            nc.sync.dma_start(out=ids_sb[:], in_=d_ids[:])

            whA = cpool.tile([128, 128], F16, name="whA")
            nc.sync.dma_start(out=whA[:], in_=d_whA[:])
            wB2 = cpool.tile([128, 64], F16, name="wB2")
            nc.sync.dma_start(out=wB2[:], in_=d_wB2[:])
            wB3 = cpool.tile([128, 64], F16, name="wB3")
            nc.sync.dma_start(out=wB3[:], in_=d_wB3[:])
            wX2 = cpool.tile([128, 64], F16, name="wX2")
            nc.sync.dma_start(out=wX2[:], in_=d_wX2[:])
            wX3 = cpool.tile([128, 64], F16, name="wX3")
            nc.sync.dma_start(out=wX3[:], in_=d_wX3[:])
            wiA = [cpool.tile([128, 128], F16, name=f"wiA{k}") for k in range(2)]
            wiB = [cpool.tile([128, 128], F16, name=f"wiB{k}") for k in range(2)]
            for k in range(2):
                nc.sync.dma_start(out=wiA[k][:], in_=d_wiA[k * 128:(k + 1) * 128, :])
                nc.sync.dma_start(out=wiB[k][:], in_=d_wiB[k * 128:(k + 1) * 128, :])
            biasA = cpool.tile([128, 1], F32, name="biasA")
            nc.sync.dma_start(out=biasA[:], in_=d_biasA[:])
            biasB = cpool.tile([128, 1], F32, name="biasB")
            nc.sync.dma_start(out=biasB[:], in_=d_biasB[:])

            ident16 = cpool.tile([128, 128], F16)
            make_identity(nc, ident16[:])

            wo16 = []
            for k, d_wo in enumerate((d_wo0, d_wo1)):
                wo = cpool.tile([128, VS], F16, name=f"wo16_{k}")
                nc.sync.dma_start(out=wo[:], in_=d_wo[:])
                wo16.append(wo)

            # fp16 history tiles, one per token tile; col = b*8 + c
            ht0 = [hpool.tile([128, 128], F16, tag="ht0", bufs=NT,
                              name=f"ht0_{g}") for g in range(NT)]
            ht1 = [hpool.tile([128, 128], F16, tag="ht1", bufs=NT,
                              name=f"ht1_{g}") for g in range(NT)]
            # tile 31's pad column (c=7) is read by the projection
            nc.vector.memset(ht0[NT - 1][:], 0.0)
            nc.vector.memset(ht1[NT - 1][:], 0.0)

            def hv(ht_g, r0, r1, c):
                # [r1-r0, 16] column view of step slot c (stride 8, offset c)
                return ht_g[r0:r1].rearrange("p (b t) -> p b t", t=8)[:, :, c]

            # ---------------- phase A: gather -> embT -> U in PSUM ----------
            bankA = {}
            bankB = {}
            gth_tiles = {}
            embt_tiles = {}

            def issue_gather(g):
                gth = wpool.tile([128, E], F32, tag="gather", bufs=6,
                                 name=f"gth_{g}")
                nc.gpsimd.indirect_dma_start(
                    out=gth[:], out_offset=None, in_=d_emb[:],
                    in_offset=bass.IndirectOffsetOnAxis(ap=ids_sb[:, g:g + 1], axis=0),
                )
                gth_tiles[g] = gth

            g16_tiles = {}

            def cast_emb(g):
                # fp32 -> fp16 on gpsimd (~1us); issued well before the PE
                # transposes that read it
                gth = gth_tiles.pop(g)
                g16 = wpool.tile([128, E], F16, tag="g16", bufs=2, name=f"g16_{g}")
                nc.gpsimd.tensor_copy(g16[:], gth[:])
                g16_tiles[g] = g16

            def prep_embt(g):
                # transpose on the PE, copy out on DVE
                g16 = g16_tiles.pop(g)
                embt = []
                for k in range(2):
                    tp = psum.tile([128, 128], F16, tag="tp", bufs=2, space="PSUM",
                                   name=f"tp_{g}_{k}")
                    nc.tensor.transpose(
                        out=tp[:], in_=g16[:, k * 128:(k + 1) * 128],
                        identity=ident16[:])
                    et = wpool.tile([128, 128], F16, tag=f"embt{k}", bufs=3,
                                    name=f"et_{g}_{k}")
                    nc.vector.tensor_copy(et[:], tp[:])
                    embt.append(et)
                embt_tiles[g] = embt

            def u_mms(g):
                embt = embt_tiles.pop(g)
                # One shared PSUM bank: cols 0:128 = blocks 0,1 (all 8 step
                # slots), cols 128:160 = blocks 3,2 at step slots c=0 / c=4.
                # The FIRST matmul's start=True marks the whole 2KB zero
                # region pending-zero, so every other matmul accumulates
                # with start=False.
                ab = psum.tile([128, 160], F32, tag="bankAB", bufs=2,
                               space="PSUM", name=f"bankAB_{g}")
                for k in range(2):
                    src = embt[k][:].rearrange("p (c2 r) -> p c2 r", c2=2)
                    nc.tensor.matmul(out=ab[:, 128:160], lhsT=wiB[k][:],
                                     rhs=src[:, :, 0:16],
                                     start=(k == 0), stop=(k == 1))
                for k in range(2):
                    nc.tensor.matmul(out=ab[:, 0:128], lhsT=wiA[k][:],
                                     rhs=embt[k][:],
                                     start=False, stop=True,
                                     skip_group_check=True)
                bankA[g] = ab
                bankB[g] = ab

            for g0 in range(4):
                issue_gather(g0)
            cast_emb(0)
            cast_emb(1)
            prep_embt(0)
            cast_emb(2)
            prep_embt(1)
            u_mms(0)

            # ---------------- projection pacing ----------------
            from collections import deque
            proj_q = deque()   # pending (g, unit) items; unit = (p, k, vc)
            ob_tiles = {}
            done_chunks = {}

            def enqueue_proj(g):
                # 8 units of 2 matmuls each; k0 units start a pair of PSUM
                # banks, k1 units finish + drain them
                for p in range(2):
                    for pair in range(2):
                        for k in range(2):
                            proj_q.append((g, p, pair, k))

            pp_banks = {}

            def emit_proj_unit():
                if not proj_q:
                    return
                g, p, pair, k = proj_q.popleft()
                if g not in ob_tiles:
                    ob_tiles[g] = opool.tile([128, VS], F16, tag="ob",
                                             name=f"ob_{g}")
                    done_chunks[g] = 0
                ht_g = ht0[g] if k == 0 else ht1[g]
                drains = []
                for vc in (2 * pair, 2 * pair + 1):
                    col = p * 2000 + vc * VC
                    if k == 0:
                        pp = psum.tile([128, VC], F32, tag="pp", bufs=2,
                                       space="PSUM", name=f"pp_{g}_{p}_{vc}")
                        pp_banks[(g, p, vc)] = pp
                        nc.tensor.matmul(out=pp[:], lhsT=ht_g[:],
                                         rhs=wo16[0][:, col:col + VC],
                                         start=True, stop=False)
                    else:
                        pp = pp_banks.pop((g, p, vc))
                        nc.tensor.matmul(out=pp[:], lhsT=ht_g[:],
                                         rhs=wo16[1][:, col:col + VC],
                                         start=False, stop=True)
                        drains.append((col, pp))
                for col, pp in drains:
                    # all drains on DVE: ACT must stay clear for the chain
                    # tanhs, gpsimd has no PSUM access
                    nc.vector.tensor_copy(ob_tiles[g][:, col:col + VC], pp[:])
                done_chunks[g] += len(drains)
                if done_chunks[g] == 8:
                    ob = ob_tiles.pop(g)
                    nc.sync.dma_start(out=d_out[g * 128:(g + 1) * 128, :],
                                      in_=ob[:])

            # ---------------- serial chains ----------------
            # per-step emission; chain1/2/3 are slotted to lag chain0.
            for t in range(T):
                g, c = divmod(t, 8)

                if c == 0 and g + 4 < NT:
                    issue_gather(g + 4)

                # --- chain0 (block0, every step) ---
                dst0 = hv(ht0[g], 0, 64, c)
                if t == 0:
                    nc.scalar.activation(dst0, bankA[0][0:64, 0:16], TANH,
                                         bias=biasA[0:64])
                else:
                    src = hv(ht0[g - 1], 0, 64, 7) if c == 0 else \
                        hv(ht0[g], 0, 64, c - 1)
                    nc.tensor.matmul(out=bankA[g][0:64, c * 16:(c + 1) * 16],
                                     lhsT=whA[0:64, 0:64], rhs=src,
                                     start=False, stop=True,
                                     skip_group_check=True)
                    nc.scalar.activation(dst0, bankA[g][0:64, c * 16:(c + 1) * 16],
                                         TANH, bias=biasA[0:64])

                # --- chain1 (block1, even steps) ---
                if c % 2 == 0:
                    dst1 = hv(ht0[g], 64, 128, c)
                    cc = slice(c * 16, (c + 1) * 16)
                    if t == 0:
                        nc.scalar.activation(dst1, bankA[0][64:128, 0:16], TANH,
                                             bias=biasA[64:128])
                    else:
                        self_src = hv(ht0[g], 64, 128, c - 2) if c >= 2 else \
                            hv(ht0[g - 1], 64, 128, 6)
                        nc.tensor.matmul(out=bankA[g][64:128, cc],
                                         lhsT=whA[64:128, 64:128], rhs=self_src,
                                         start=False, stop=True,
                                         skip_group_check=True)
                        nc.scalar.activation(dst1, bankA[g][64:128, cc], TANH,
                                             bias=biasA[64:128])
                    # held value for the odd step c+1 (off critical path)
                    v1 = ht0[g][64:128].rearrange("p (b t) -> p b t", t=8)
                    nc.gpsimd.tensor_copy(v1[:, :, c + 1], dst1)

                # --- cross block0 -> block1 for col c+1 (odd c) ---
                if c in (1, 3, 5) and t + 1 < T:
                    cc1 = slice((c + 1) * 16, (c + 2) * 16)
                    nc.tensor.matmul(out=bankA[g][64:128, cc1],
                                     lhsT=whA[0:64, 64:128],
                                     rhs=hv(ht0[g], 0, 64, c),
                                     start=False, stop=True,
                                     skip_group_check=True)
                if c == 7 and g + 1 < NT:
                    # cross into next tile's col 0
                    nc.tensor.matmul(out=bankA[g + 1][64:128, 0:16],
                                     lhsT=whA[0:64, 64:128],
                                     rhs=hv(ht0[g], 0, 64, 7),
                                     start=False, stop=True,
                                     skip_group_check=True)

                # --- chain3 (block3, t%8==0), slotted at c==1 ---
                if c == 1:
                    dst3 = hv(ht1[g], 0, 64, 0)
                    if g == 0:
                        nc.scalar.activation(dst3, bankB[0][0:64, 128:144], TANH,
                                             bias=biasB[0:64])
                    else:
                        # cross from blocks 0,1 at t-1
                        nc.tensor.matmul(out=bankB[g][0:64, 128:144],
                                         lhsT=wX3[:],
                                         rhs=ht0[g - 1][:].rearrange(
                                             "p (b t) -> p b t", t=8)[:, :, 7],
                                         start=False, stop=True,
                                         skip_group_check=True)
                        # self W33 + W23 (block2 state held at col 4)
                        nc.tensor.matmul(out=bankB[g][0:64, 128:144],
                                         lhsT=wB3[:],
                                         rhs=ht1[g - 1][:].rearrange(
                                             "p (b t) -> p b t", t=8)[:, :, 4],
                                         start=False, stop=True,
                                         skip_group_check=True)
                        nc.scalar.activation(dst3, bankB[g][0:64, 128:144], TANH,
                                             bias=biasB[0:64])
                    v3 = ht1[g][0:64].rearrange("p (b t) -> p b t", t=8)
                    nc.gpsimd.tensor_copy(
                        v3[:, :, 1:8],
                        dst3[:, :, None].to_broadcast([64, B, 7]))

                if c == 1 and g + 2 < NT:
                    prep_embt(g + 2)
                if c == 6 and g + 3 < NT:
                    cast_emb(g + 3)
                if c == 5 and g + 1 < NT:
                    u_mms(g + 1)

                # --- chain2 (block2, t%4==0), slotted at c==2 and c==5 ---
                if c == 2 or c == 5:
                    cs = 0 if c == 2 else 4          # step slot being computed
                    bb_cols = slice(128, 144) if cs == 0 else slice(144, 160)
                    dst2 = hv(ht1[g], 64, 128, cs)
                    if t <= 2:
                        nc.scalar.activation(dst2, bankB[0][64:128, bb_cols],
                                             TANH, bias=biasB[64:128])
                    else:
                        # cross from blocks 0,1 at t-1
                        xsrc = ht0[g - 1][:].rearrange(
                            "p (b t) -> p b t", t=8)[:, :, 7] if cs == 0 else \
                            ht0[g][:].rearrange(
                                "p (b t) -> p b t", t=8)[:, :, 3]
                        nc.tensor.matmul(out=bankB[g][64:128, bb_cols],
                                         lhsT=wX2[:], rhs=xsrc,
                                         start=False, stop=True,
                                         skip_group_check=True)
                        self_src = hv(ht1[g - 1], 64, 128, 4) if cs == 0 else \
                            hv(ht1[g], 64, 128, 0)
                        nc.tensor.matmul(out=bankB[g][64:128, bb_cols],
                                         lhsT=wB2[64:128, :], rhs=self_src,
                                         start=False, stop=True,
                                         skip_group_check=True)
                        nc.scalar.activation(dst2, bankB[g][64:128, bb_cols],
                                             TANH, bias=biasB[64:128])
                    span = 3 if cs == 0 else min(3, T - t + 1)
                    v2 = ht1[g][64:128].rearrange("p (b t) -> p b t", t=8)
                    nc.gpsimd.tensor_copy(
                        v2[:, :, cs + 1:cs + 1 + span],
                        dst2[:, :, None].to_broadcast([64, B, span]))

                # --- projection pacing: 1 unit (2 matmuls) per step ---
                if c == 7 and g >= PROJ_LAG:
                    enqueue_proj(g - PROJ_LAG)
                emit_proj_unit()

            # flush remaining projection work (tiles whose c==7 enqueue
            # never fired: the last PROJ_LAG tiles plus tile NT-1 itself)
            for g in range(NT - PROJ_LAG - 1, NT):
                enqueue_proj(g)
            while proj_q:
                emit_proj_unit()

            if DEBUG_HT:
                for g in range(NT):
                    # dbg row = g*128 + unit_partition, col = token slot b*8+c
                    nc.sync.dma_start(out=d_dbg[g * 128:(g + 1) * 128, 0:128],
                                      in_=ht0[g][:])
                    nc.sync.dma_start(out=d_dbg[g * 128:(g + 1) * 128, 128:256],
                                      in_=ht1[g][:])

    nc.finalize()
    return nc


_NC_CACHE = None
TRACE = False        # set by test harness to capture an NTFF profile
TRACE_KW = {}
LAST_RESULT = None   # BassKernelResults of the most recent run
DEBUG_HT = False     # add a debug output with the recorded h history


def kernel(x, x_sl, embedding, Wi, Wh, bi, bh, Wo):
    global _NC_CACHE, LAST_RESULT
    if _NC_CACHE is None:
        _NC_CACHE = build_program()
    nc = _NC_CACHE

    x = np.asarray(x)
    ids = np.ascontiguousarray(x[:, :T].T).reshape(-1)  # n = t*B + b
    ids_pad = np.zeros(128 * NT, np.int32)
    ids_pad[:B * T] = ids
    ids_dev = np.ascontiguousarray(ids_pad.reshape(NT, 128).T)

    embedding = np.ascontiguousarray(np.asarray(embedding, np.float32))
    Wh16 = np.asarray(Wh, np.float16)
    Wi16 = np.asarray(Wi, np.float16)
    biasv = (np.asarray(bi, np.float32) + np.asarray(bh, np.float32))
    Wo16 = np.asarray(Wo, np.float16)

    whA_h = np.ascontiguousarray(Wh16[0:128, 0:128])
    wB2_h = np.zeros((128, 64), np.float16)
    wB2_h[64:128] = Wh16[128:192, 128:192]
    wB3_h = np.zeros((128, 64), np.float16)
    wB3_h[0:64] = Wh16[192:256, 192:256]
    wB3_h[64:128] = Wh16[128:192, 192:256]
    wX2_h = np.ascontiguousarray(Wh16[0:128, 128:192])
    wX3_h = np.ascontiguousarray(Wh16[0:128, 192:256])
    wiA_h = np.ascontiguousarray(Wi16[:, 0:128])
    wiB_h = np.ascontiguousarray(
        np.concatenate([Wi16[:, 192:256], Wi16[:, 128:192]], axis=1))
    biasA_h = np.ascontiguousarray(biasv[0:128].reshape(128, 1))
    biasB_h = np.ascontiguousarray(
        np.concatenate([biasv[192:256], biasv[128:192]]).reshape(128, 1))

    in_maps = []
    for cidx in range(NCORES):
        sl = slice(cidx * VS, (cidx + 1) * VS)
        in_maps.append({
            "tok_ids": ids_dev,
            "embedding": embedding,
            "whA": whA_h, "wB2": wB2_h, "wB3": wB3_h,
            "wX2": wX2_h, "wX3": wX3_h,
            "wiA": wiA_h, "wiB": wiB_h,
            "biasA": biasA_h, "biasB": biasB_h,
            "wo0": np.ascontiguousarray(Wo16[0:128, sl]),
            "wo1": np.ascontiguousarray(
                np.concatenate([Wo16[192:256, sl], Wo16[128:192, sl]], axis=0)),
        })

    res = run_bass_kernel_spmd(nc, in_maps, core_ids=list(range(NCORES)),
                               trace=TRACE, **TRACE_KW)
    LAST_RESULT = res
    raw = np.concatenate([r["out"] for r in res.results], axis=1)  # [4096, V]
    out = raw.reshape(NT, B, 8, V).transpose(1, 0, 2, 3).reshape(B, NT * 8, V)
    return out[:, :T].astype(np.float32)


# revision 21
# speedup vs baseline: 1.2961x; 1.2517x over previous
"""CWRNN language-model kernel for 8 Trainium2 NeuronCores.

Strategy (vocab-sharded output projection, v2):
  - Each core owns Wo[:, c*4000:(c+1)*4000] and writes its logits slice in
    fp16 (tolerance 2e-2 >> fp16 rounding) -> halves the HBM write volume
    that dominated v1.
  - The clockwork mask is block-triangular: block0 (period 1, units 0:64)
    is fed only by itself, so the serial critical path is a 64-unit RNN.
    Blocks 1-3 run as separate serial chains lagging block0, with their
    cross-block input terms batched as per-tile matmuls over the recorded
    history tiles.
  - U = emb @ Wi is accumulated directly into PSUM banks (phase A); chain
    matmuls accumulate h @ Whh on top (start=False) and tanh reads PSUM
    with the bias folded into the activation -> each chain step is exactly
    matmul -> tanh, and tanh writes straight into the fp16 history tile
    column the next step's matmul reads (no DVE add, no Pool copy on the
    critical path).
  - Projection pairs two 500-col chunks per weight load, paced one unit
    per chain step two tiles behind the recurrence; all PSUM drains stay
    on DVE (ACT is reserved for the serial tanh chain; gpsimd has no PSUM
    access); one fp16 output DMA per tile on the sync queue.
  - Pacing note: the PE sustains only ~50% of its 2.4GHz peak (power
    throttle).  Denser projection pacing (more PSUM banks, bigger bursts)
    measurably LOWERS throughput once the throttle engages, so the
    drain-coupled pp=2 pacing is deliberate.
"""

import sys

sys.path.insert(0, "/opt/trn_rl_repo")

import numpy as np

import concourse.bass as bass
import concourse.mybir as mybir
import concourse.tile as tile
from concourse import bacc
from concourse import bass_utils as _bass_utils
from concourse.bass_utils import run_bass_kernel_spmd
from concourse.masks import make_identity

# note: walrus's --enable-ldw-opt pass crashes codegen on this toolchain
# (visitInstLdweights), so per-matmul LDWEIGHTS reloads are unavoidable

F32 = mybir.dt.float32
F16 = mybir.dt.float16
I32 = mybir.dt.int32
TANH = mybir.ActivationFunctionType.Tanh

B = 16
T = 255           # x[:, :-1]
E = 256
NH = 256
V = 32000
NCORES = 8
VS = V // NCORES  # 4000 vocab columns per core
NT = 32           # token tiles of 8 steps (tile 31 has 7 real steps)
VC = 500          # vocab chunk per PSUM bank
PROJ_LAG = 2      # tiles between recurrence and projection start


def build_program():
    nc = bacc.Bacc(target_bir_lowering=False)

    d_ids = nc.dram_tensor("tok_ids", [128, NT], I32, kind="ExternalInput")
    d_emb = nc.dram_tensor("embedding", [V + 1, E], F32, kind="ExternalInput")
    d_whA = nc.dram_tensor("whA", [128, 128], F16, kind="ExternalInput")
    d_wB2 = nc.dram_tensor("wB2", [128, 64], F16, kind="ExternalInput")
    d_wB3 = nc.dram_tensor("wB3", [128, 64], F16, kind="ExternalInput")
    d_wX2 = nc.dram_tensor("wX2", [128, 64], F16, kind="ExternalInput")
    d_wX3 = nc.dram_tensor("wX3", [128, 64], F16, kind="ExternalInput")
    d_wiA = nc.dram_tensor("wiA", [256, 128], F16, kind="ExternalInput")
    d_wiB = nc.dram_tensor("wiB", [256, 128], F16, kind="ExternalInput")
    d_biasA = nc.dram_tensor("biasA", [128, 1], F32, kind="ExternalInput")
    d_biasB = nc.dram_tensor("biasB", [128, 1], F32, kind="ExternalInput")
    d_wo0 = nc.dram_tensor("wo0", [128, VS], F16, kind="ExternalInput")
    d_wo1 = nc.dram_tensor("wo1", [128, VS], F16, kind="ExternalInput")
    # raw token-major output: row g*128 + b*8 + c  <->  logits[b, g*8+c]
    d_out = nc.dram_tensor("out", [NT * 128, VS], F16, kind="ExternalOutput")
    d_dbg = None
    if DEBUG_HT:
        d_dbg = nc.dram_tensor("dbg_ht", [NT * 128, 256], F16,
                               kind="ExternalOutput")

    with tile.TileContext(nc) as tc:
        with tc.tile_pool(name="const", bufs=1) as cpool, \
             tc.tile_pool(name="hist", bufs=1) as hpool, \
             tc.tile_pool(name="obuf", bufs=3) as opool, \
             tc.tile_pool(name="work", bufs=3) as wpool, \
             tc.tile_pool(name="psum", bufs=2, space="PSUM") as psum:

            # ---------------- constants and weights ----------------
            ids_sb = cpool.tile([128, NT], I32)
            nc.sync.dma_start(out=ids_sb[:], in_=d_ids[:])

            whA = cpool.tile([128, 128], F16, name="whA")
            nc.sync.dma_start(out=whA[:], in_=d_whA[:])
            wB2 = cpool.tile([128, 64], F16, name="wB2")
            nc.sync.dma_start(out=wB2[:], in_=d_wB2[:])
            wB3 = cpool.tile([128, 64], F16, name="wB3")
            nc.sync.dma_start(out=wB3[:], in_=d_wB3[:])
            wX2 = cpool.tile([128, 64], F16, name="wX2")
            nc.sync.dma_start(out=wX2[:], in_=d_wX2[:])
            wX3 = cpool.tile([128, 64], F16, name="wX3")
            nc.sync.dma_start(out=wX3[:], in_=d_wX3[:])
            wiA = [cpool.tile([128, 128], F16, name=f"wiA{k}") for k in range(2)]
            wiB = [cpool.tile([128, 128], F16, name=f"wiB{k}") for k in range(2)]
            for k in range(2):
                nc.sync.dma_start(out=wiA[k][:], in_=d_wiA[k * 128:(k + 1) * 128, :])
                nc.sync.dma_start(out=wiB[k][:], in_=d_wiB[k * 128:(k + 1) * 128, :])
            biasA = cpool.tile([128, 1], F32, name="biasA")
            nc.sync.dma_start(out=biasA[:], in_=d_biasA[:])
            biasB = cpool.tile([128, 1], F32, name="biasB")
            nc.sync.dma_start(out=biasB[:], in_=d_biasB[:])

            ident16 = cpool.tile([128, 128], F16)
            make_identity(nc, ident16[:])

            wo16 = []
            for k, d_wo in enumerate((d_wo0, d_wo1)):
                wo = cpool.tile([128, VS], F16, name=f"wo16_{k}")
                nc.sync.dma_start(out=wo[:], in_=d_wo[:])
                wo16.append(wo)

            # fp16 history tiles, one per token tile; col = b*8 + c
            ht0 = [hpool.tile([128, 128], F16, tag="ht0", bufs=NT,
                              name=f"ht0_{g}") for g in range(NT)]
            ht1 = [hpool.tile([128, 128], F16, tag="ht1", bufs=NT,
                              name=f"ht1_{g}") for g in range(NT)]
            # tile 31's pad column (c=7) is read by the projection
            nc.vector.memset(ht0[NT - 1][:], 0.0)
            nc.vector.memset(ht1[NT - 1][:], 0.0)

            def hv(ht_g, r0, r1, c):
                # [r1-r0, 16] column view of step slot c (stride 8, offset c)
                return ht_g[r0:r1].rearrange("p (b t) -> p b t", t=8)[:, :, c]

            # ---------------- phase A: gather -> embT -> U in PSUM ----------
            bankA = {}
            bankB = {}
            gth_tiles = {}
            embt_tiles = {}

            def issue_gather(g):
                gth = wpool.tile([128, E], F32, tag="gather", bufs=6,
                                 name=f"gth_{g}")
                nc.gpsimd.indirect_dma_start(
                    out=gth[:], out_offset=None, in_=d_emb[:],
                    in_offset=bass.IndirectOffsetOnAxis(ap=ids_sb[:, g:g + 1], axis=0),
                )
                gth_tiles[g] = gth

            g16_tiles = {}

            def cast_emb(g):
                # fp32 -> fp16 on gpsimd (~1us); issued well before the PE
                # transposes that read it
                gth = gth_tiles.pop(g)
                g16 = wpool.tile([128, E], F16, tag="g16", bufs=2, name=f"g16_{g}")
                nc.gpsimd.tensor_copy(g16[:], gth[:])
                g16_tiles[g] = g16

            def prep_embt(g):
                # transpose on the PE, copy out on DVE
                g16 = g16_tiles.pop(g)
                embt = []
                for k in range(2):
                    tp = psum.tile([128, 128], F16, tag="tp", bufs=2, space="PSUM",
                                   name=f"tp_{g}_{k}")
                    nc.tensor.transpose(
                        out=tp[:], in_=g16[:, k * 128:(k + 1) * 128],
                        identity=ident16[:])
                    et = wpool.tile([128, 128], F16, tag=f"embt{k}", bufs=3,
                                    name=f"et_{g}_{k}")
                    nc.vector.tensor_copy(et[:], tp[:])
                    embt.append(et)
                embt_tiles[g] = embt

            def u_mms(g):
                embt = embt_tiles.pop(g)
                # U for blocks 0,1: all 128 cols (col = c*16 + b, t-major)
                ba = psum.tile([128, 128], F32, tag="bankA", bufs=2,
                               space="PSUM", name=f"bankA_{g}")
                for k in range(2):
                    nc.tensor.matmul(out=ba[:], lhsT=wiA[k][:], rhs=embt[k][:],
                                     start=(k == 0), stop=(k == 1))
                bankA[g] = ba
                # U for blocks 3,2 (rows 0:64 = block3, 64:128 = block2) at
                # step slots c=0 / c=4; single start/stop pair per bank
                # (start marks the whole 2KB zero region pending-zero)
                bb = psum.tile([128, 32], F32, tag="bankB", bufs=2,
                               space="PSUM", name=f"bankB_{g}")
                for k in range(2):
                    src = embt[k][:].rearrange("p (c2 r) -> p c2 r", c2=2)
                    nc.tensor.matmul(out=bb[:, 0:32], lhsT=wiB[k][:],
                                     rhs=src[:, :, 0:16],
                                     start=(k == 0), stop=(k == 1))
                bankB[g] = bb

            for g0 in range(4):
                issue_gather(g0)
            cast_emb(0)
            cast_emb(1)
            prep_embt(0)
            cast_emb(2)
            prep_embt(1)
            u_mms(0)

            # ---------------- projection pacing ----------------
            from collections import deque
            proj_q = deque()   # pending (g, unit) items; unit = (p, k, vc)
            ob_tiles = {}
            done_chunks = {}

            def enqueue_proj(g):
                # 8 units of 2 matmuls each; k0 units start a pair of PSUM
                # banks, k1 units finish + drain them
                for p in range(2):
                    for pair in range(2):
                        for k in range(2):
                            proj_q.append((g, p, pair, k))

            pp_banks = {}

            def emit_proj_unit():
                if not proj_q:
                    return
                g, p, pair, k = proj_q.popleft()
                if g not in ob_tiles:
                    ob_tiles[g] = opool.tile([128, VS], F16, tag="ob",
                                             name=f"ob_{g}")
                    done_chunks[g] = 0
                ht_g = ht0[g] if k == 0 else ht1[g]
                drains = []
                for vc in (2 * pair, 2 * pair + 1):
                    col = p * 2000 + vc * VC
                    if k == 0:
                        pp = psum.tile([128, VC], F32, tag="pp", bufs=2,
                                       space="PSUM", name=f"pp_{g}_{p}_{vc}")
                        pp_banks[(g, p, vc)] = pp
                        nc.tensor.matmul(out=pp[:], lhsT=ht_g[:],
                                         rhs=wo16[0][:, col:col + VC],
                                         start=True, stop=False)
                    else:
                        pp = pp_banks.pop((g, p, vc))
                        nc.tensor.matmul(out=pp[:], lhsT=ht_g[:],
                                         rhs=wo16[1][:, col:col + VC],
                                         start=False, stop=True)
                        drains.append((col, pp))
                for col, pp in drains:
                    # all drains on DVE: ACT must stay clear for the chain
                    # tanhs, gpsimd has no PSUM access
                    nc.vector.tensor_copy(ob_tiles[g][:, col:col + VC], pp[:])
                done_chunks[g] += len(drains)
                if done_chunks[g] == 8:
                    ob = ob_tiles.pop(g)
                    nc.sync.dma_start(out=d_out[g * 128:(g + 1) * 128, :],
                                      in_=ob[:])

            # ---------------- serial chains ----------------
            # per-step emission; chain1/2/3 are slotted to lag chain0.
            for t in range(T):
                g, c = divmod(t, 8)

                if c == 0 and g + 4 < NT:
                    issue_gather(g + 4)

                # --- chain0 (block0, every step) ---
                dst0 = hv(ht0[g], 0, 64, c)
                if t == 0:
                    nc.scalar.activation(dst0, bankA[0][0:64, 0:16], TANH,
                                         bias=biasA[0:64])
                else:
                    src = hv(ht0[g - 1], 0, 64, 7) if c == 0 else \
                        hv(ht0[g], 0, 64, c - 1)
                    nc.tensor.matmul(out=bankA[g][0:64, c * 16:(c + 1) * 16],
                                     lhsT=whA[0:64, 0:64], rhs=src,
                                     start=False, stop=True,
                                     skip_group_check=True)
                    nc.scalar.activation(dst0, bankA[g][0:64, c * 16:(c + 1) * 16],
                                         TANH, bias=biasA[0:64])

                # --- chain1 (block1, even steps) ---
                if c % 2 == 0:
                    dst1 = hv(ht0[g], 64, 128, c)
                    cc = slice(c * 16, (c + 1) * 16)
                    if t == 0:
                        nc.scalar.activation(dst1, bankA[0][64:128, 0:16], TANH,
                                             bias=biasA[64:128])
                    else:
                        self_src = hv(ht0[g], 64, 128, c - 2) if c >= 2 else \
                            hv(ht0[g - 1], 64, 128, 6)
                        nc.tensor.matmul(out=bankA[g][64:128, cc],
                                         lhsT=whA[64:128, 64:128], rhs=self_src,
                                         start=False, stop=True,
                                         skip_group_check=True)
                        nc.scalar.activation(dst1, bankA[g][64:128, cc], TANH,
                                             bias=biasA[64:128])
                    # held value for the odd step c+1 (off critical path)
                    v1 = ht0[g][64:128].rearrange("p (b t) -> p b t", t=8)
                    nc.gpsimd.tensor_copy(v1[:, :, c + 1], dst1)

                # --- cross block0 -> block1 for col c+1 (odd c) ---
                if c in (1, 3, 5) and t + 1 < T:
                    cc1 = slice((c + 1) * 16, (c + 2) * 16)
                    nc.tensor.matmul(out=bankA[g][64:128, cc1],
                                     lhsT=whA[0:64, 64:128],
                                     rhs=hv(ht0[g], 0, 64, c),
                                     start=False, stop=True,
                                     skip_group_check=True)
                if c == 7 and g + 1 < NT:
                    # cross into next tile's col 0
                    nc.tensor.matmul(out=bankA[g + 1][64:128, 0:16],
                                     lhsT=whA[0:64, 64:128],
                                     rhs=hv(ht0[g], 0, 64, 7),
                                     start=False, stop=True,
                                     skip_group_check=True)

                # --- chain3 (block3, t%8==0), slotted at c==1 ---
                if c == 1:
                    dst3 = hv(ht1[g], 0, 64, 0)
                    if g == 0:
                        nc.scalar.activation(dst3, bankB[0][0:64, 0:16], TANH,
                                             bias=biasB[0:64])
                    else:
                        # cross from blocks 0,1 at t-1
                        nc.tensor.matmul(out=bankB[g][0:64, 0:16],
                                         lhsT=wX3[:],
                                         rhs=ht0[g - 1][:].rearrange(
                                             "p (b t) -> p b t", t=8)[:, :, 7],
                                         start=False, stop=True,
                                         skip_group_check=True)
                        # self W33 + W23 (block2 state held at col 4)
                        nc.tensor.matmul(out=bankB[g][0:64, 0:16],
                                         lhsT=wB3[:],
                                         rhs=ht1[g - 1][:].rearrange(
                                             "p (b t) -> p b t", t=8)[:, :, 4],
                                         start=False, stop=True,
                                         skip_group_check=True)
                        nc.scalar.activation(dst3, bankB[g][0:64, 0:16], TANH,
                                             bias=biasB[0:64])
                    v3 = ht1[g][0:64].rearrange("p (b t) -> p b t", t=8)
                    nc.gpsimd.tensor_copy(
                        v3[:, :, 1:8],
                        dst3[:, :, None].to_broadcast([64, B, 7]))

                if c == 1 and g + 2 < NT:
                    prep_embt(g + 2)
                if c == 6 and g + 3 < NT:
                    cast_emb(g + 3)
                if c == 5 and g + 1 < NT:
                    u_mms(g + 1)

                # --- chain2 (block2, t%4==0), slotted at c==2 and c==5 ---
                if c == 2 or c == 5:
                    cs = 0 if c == 2 else 4          # step slot being computed
                    bb_cols = slice(0, 16) if cs == 0 else slice(16, 32)
                    dst2 = hv(ht1[g], 64, 128, cs)
                    if t <= 2:
                        nc.scalar.activation(dst2, bankB[0][64:128, bb_cols],
                                             TANH, bias=biasB[64:128])
                    else:
                        # cross from blocks 0,1 at t-1
                        xsrc = ht0[g - 1][:].rearrange(
                            "p (b t) -> p b t", t=8)[:, :, 7] if cs == 0 else \
                            ht0[g][:].rearrange(
                                "p (b t) -> p b t", t=8)[:, :, 3]
                        nc.tensor.matmul(out=bankB[g][64:128, bb_cols],
                                         lhsT=wX2[:], rhs=xsrc,
                                         start=False, stop=True,
                                         skip_group_check=True)
                        self_src = hv(ht1[g - 1], 64, 128, 4) if cs == 0 else \
                            hv(ht1[g], 64, 128, 0)
                        nc.tensor.matmul(out=bankB[g][64:128, bb_cols],
                                         lhsT=wB2[64:128, :], rhs=self_src,
                                         start=False, stop=True,
                                         skip_group_check=True)
                        nc.scalar.activation(dst2, bankB[g][64:128, bb_cols],
                                             TANH, bias=biasB[64:128])
                    span = 3 if cs == 0 else min(3, T - t + 1)
                    v2 = ht1[g][64:128].rearrange("p (b t) -> p b t", t=8)
                    nc.gpsimd.tensor_copy(
                        v2[:, :, cs + 1:cs + 1 + span],
                        dst2[:, :, None].to_broadcast([64, B, span]))

                # --- projection pacing: 1 unit (2 matmuls) per step ---
                if c == 7 and g >= PROJ_LAG:
                    enqueue_proj(g - PROJ_LAG)
                emit_proj_unit()

            # flush remaining projection work (tiles whose c==7 enqueue
            # never fired: the last PROJ_LAG tiles plus tile NT-1 itself)
            for g in range(NT - PROJ_LAG - 1, NT):
                enqueue_proj(g)
            while proj_q:
                emit_proj_unit()

            if DEBUG_HT:
                for g in range(NT):
                    # dbg row = g*128 + unit_partition, col = token slot b*8+c
                    nc.sync.dma_start(out=d_dbg[g * 128:(g + 1) * 128, 0:128],
                                      in_=ht0[g][:])
                    nc.sync.dma_start(out=d_dbg[g * 128:(g + 1) * 128, 128:256],
                                      in_=ht1[g][:])

    nc.finalize()
    return nc


_NC_CACHE = None
TRACE = False        # set by test harness to capture an NTFF profile
TRACE_KW = {}
LAST_RESULT = None   # BassKernelResults of the most recent run
DEBUG_HT = False     # add a debug output with the recorded h history


def kernel(x, x_sl, embedding, Wi, Wh, bi, bh, Wo):
    global _NC_CACHE, LAST_RESULT
    if _NC_CACHE is None:
        _NC_CACHE = build_program()
    nc = _NC_CACHE

    x = np.asarray(x)
    ids = np.ascontiguousarray(x[:, :T].T).reshape(-1)  # n = t*B + b
    ids_pad = np.zeros(128 * NT, np.int32)
    ids_pad[:B * T] = ids
    ids_dev = np.ascontiguousarray(ids_pad.reshape(NT, 128).T)

    embedding = np.ascontiguousarray(np.asarray(embedding, np.float32))
    Wh16 = np.asarray(Wh, np.float16)
    Wi16 = np.asarray(Wi, np.float16)
    biasv = (np.asarray(bi, np.float32) + np.asarray(bh, np.float32))
    Wo16 = np.asarray(Wo, np.float16)

    whA_h = np.ascontiguousarray(Wh16[0:128, 0:128])
    wB2_h = np.zeros((128, 64), np.float16)
    wB2_h[64:128] = Wh16[128:192, 128:192]
    wB3_h = np.zeros((128, 64), np.float16)
    wB3_h[0:64] = Wh16[192:256, 192:256]
    wB3_h[64:128] = Wh16[128:192, 192:256]
    wX2_h = np.ascontiguousarray(Wh16[0:128, 128:192])
    wX3_h = np.ascontiguousarray(Wh16[0:128, 192:256])
    wiA_h = np.ascontiguousarray(Wi16[:, 0:128])
    wiB_h = np.ascontiguousarray(
        np.concatenate([Wi16[:, 192:256], Wi16[:, 128:192]], axis=1))
    biasA_h = np.ascontiguousarray(biasv[0:128].reshape(128, 1))
    biasB_h = np.ascontiguousarray(
        np.concatenate([biasv[192:256], biasv[128:192]]).reshape(128, 1))

    in_maps = []
    for cidx in range(NCORES):
        sl = slice(cidx * VS, (cidx + 1) * VS)
        in_maps.append({
            "tok_ids": ids_dev,
            "embedding": embedding,
            "whA": whA_h, "wB2": wB2_h, "wB3": wB3_h,
            "wX2": wX2_h, "wX3": wX3_h,
            "wiA": wiA_h, "wiB": wiB_h,
            "biasA": biasA_h, "biasB": biasB_h,
            "wo0": np.ascontiguousarray(Wo16[0:128, sl]),
            "wo1": np.ascontiguousarray(
                np.concatenate([Wo16[192:256, sl], Wo16[128:192, sl]], axis=0)),
        })

    res = run_bass_kernel_spmd(nc, in_maps, core_ids=list(range(NCORES)),
                               trace=TRACE, **TRACE_KW)
    LAST_RESULT = res
    raw = np.concatenate([r["out"] for r in res.results], axis=1)  # [4096, V]
    out = raw.reshape(NT, B, 8, V).transpose(1, 0, 2, 3).reshape(B, NT * 8, V)
    return out[:, :T].astype(np.float32)


# revision 22
# speedup vs baseline: 1.2982x; 1.0016x over previous
"""CWRNN language-model kernel for 8 Trainium2 NeuronCores.

Strategy (vocab-sharded output projection, v2):
  - Each core owns Wo[:, c*4000:(c+1)*4000] and writes its logits slice in
    fp16 (tolerance 2e-2 >> fp16 rounding) -> halves the HBM write volume
    that dominated v1.
  - The clockwork mask is block-triangular: block0 (period 1, units 0:64)
    is fed only by itself, so the serial critical path is a 64-unit RNN.
    Blocks 1-3 run as separate serial chains lagging block0, with their
    cross-block input terms batched as per-tile matmuls over the recorded
    history tiles.
  - U = emb @ Wi is accumulated directly into PSUM banks (phase A); chain
    matmuls accumulate h @ Whh on top (start=False) and tanh reads PSUM
    with the bias folded into the activation -> each chain step is exactly
    matmul -> tanh, and tanh writes straight into the fp16 history tile
    column the next step's matmul reads (no DVE add, no Pool copy on the
    critical path).
  - Projection pairs two 500-col chunks per weight load, paced one unit
    per chain step two tiles behind the recurrence; all PSUM drains stay
    on DVE (ACT is reserved for the serial tanh chain; gpsimd has no PSUM
    access); one fp16 output DMA per tile on the sync queue.
  - Pacing note: the PE sustains only ~50% of its 2.4GHz peak (power
    throttle).  Denser projection pacing (more PSUM banks, bigger bursts)
    measurably LOWERS throughput once the throttle engages, so the
    drain-coupled pp=2 pacing is deliberate.
"""

import sys

sys.path.insert(0, "/opt/trn_rl_repo")

import numpy as np

import concourse.bass as bass
import concourse.mybir as mybir
import concourse.tile as tile
from concourse import bacc
from concourse import bass_utils as _bass_utils
from concourse.bass_utils import run_bass_kernel_spmd
from concourse.masks import make_identity

# note: walrus's --enable-ldw-opt pass crashes codegen on this toolchain
# (visitInstLdweights), so per-matmul LDWEIGHTS reloads are unavoidable

F32 = mybir.dt.float32
F16 = mybir.dt.float16
I32 = mybir.dt.int32
TANH = mybir.ActivationFunctionType.Tanh

B = 16
T = 255           # x[:, :-1]
E = 256
NH = 256
V = 32000
NCORES = 8
VS = V // NCORES  # 4000 vocab columns per core
NT = 32           # token tiles of 8 steps (tile 31 has 7 real steps)
VC = 500          # vocab chunk per PSUM bank
PROJ_LAG = 2      # tiles between recurrence and projection start


def build_program():
    nc = bacc.Bacc(target_bir_lowering=False)

    d_ids = nc.dram_tensor("tok_ids", [128, NT], I32, kind="ExternalInput")
    d_emb = nc.dram_tensor("embedding", [V + 1, E], F32, kind="ExternalInput")
    d_whA = nc.dram_tensor("whA", [128, 128], F16, kind="ExternalInput")
    d_wB2 = nc.dram_tensor("wB2", [128, 64], F16, kind="ExternalInput")
    d_wB3 = nc.dram_tensor("wB3", [128, 64], F16, kind="ExternalInput")
    d_wX2 = nc.dram_tensor("wX2", [128, 64], F16, kind="ExternalInput")
    d_wX3 = nc.dram_tensor("wX3", [128, 64], F16, kind="ExternalInput")
    d_wiA = nc.dram_tensor("wiA", [256, 128], F16, kind="ExternalInput")
    d_wiB = nc.dram_tensor("wiB", [256, 128], F16, kind="ExternalInput")
    d_biasA = nc.dram_tensor("biasA", [128, 1], F32, kind="ExternalInput")
    d_biasB = nc.dram_tensor("biasB", [128, 1], F32, kind="ExternalInput")
    d_wo0 = nc.dram_tensor("wo0", [128, VS], F16, kind="ExternalInput")
    d_wo1 = nc.dram_tensor("wo1", [128, VS], F16, kind="ExternalInput")
    # raw token-major output: row g*128 + b*8 + c  <->  logits[b, g*8+c]
    d_out = nc.dram_tensor("out", [NT * 128, VS], F16, kind="ExternalOutput")
    d_dbg = None
    if DEBUG_HT:
        d_dbg = nc.dram_tensor("dbg_ht", [NT * 128, 256], F16,
                               kind="ExternalOutput")

    with tile.TileContext(nc) as tc:
        with tc.tile_pool(name="const", bufs=1) as cpool, \
             tc.tile_pool(name="hist", bufs=1) as hpool, \
             tc.tile_pool(name="obuf", bufs=3) as opool, \
             tc.tile_pool(name="work", bufs=3) as wpool, \
             tc.tile_pool(name="psum", bufs=2, space="PSUM") as psum:

            # ---------------- constants and weights ----------------
            ids_sb = cpool.tile([128, NT], I32)
            nc.sync.dma_start(out=ids_sb[:], in_=d_ids[:])

            whA = cpool.tile([128, 128], F16, name="whA")
            nc.sync.dma_start(out=whA[:], in_=d_whA[:])
            wB2 = cpool.tile([128, 64], F16, name="wB2")
            nc.sync.dma_start(out=wB2[:], in_=d_wB2[:])
            wB3 = cpool.tile([128, 64], F16, name="wB3")
            nc.sync.dma_start(out=wB3[:], in_=d_wB3[:])
            wX2 = cpool.tile([128, 64], F16, name="wX2")
            nc.sync.dma_start(out=wX2[:], in_=d_wX2[:])
            wX3 = cpool.tile([128, 64], F16, name="wX3")
            nc.sync.dma_start(out=wX3[:], in_=d_wX3[:])
            wiA = [cpool.tile([128, 128], F16, name=f"wiA{k}") for k in range(2)]
            wiB = [cpool.tile([128, 128], F16, name=f"wiB{k}") for k in range(2)]
            for k in range(2):
                nc.sync.dma_start(out=wiA[k][:], in_=d_wiA[k * 128:(k + 1) * 128, :])
                nc.sync.dma_start(out=wiB[k][:], in_=d_wiB[k * 128:(k + 1) * 128, :])
            biasA = cpool.tile([128, 1], F32, name="biasA")
            nc.sync.dma_start(out=biasA[:], in_=d_biasA[:])
            biasB = cpool.tile([128, 1], F32, name="biasB")
            nc.sync.dma_start(out=biasB[:], in_=d_biasB[:])

            ident16 = cpool.tile([128, 128], F16)
            make_identity(nc, ident16[:])

            wo16 = []
            for k, d_wo in enumerate((d_wo0, d_wo1)):
                wo = cpool.tile([128, VS], F16, name=f"wo16_{k}")
                nc.scalar.dma_start(out=wo[:], in_=d_wo[:])
                wo16.append(wo)

            # fp16 history tiles, one per token tile; col = b*8 + c
            ht0 = [hpool.tile([128, 128], F16, tag="ht0", bufs=NT,
                              name=f"ht0_{g}") for g in range(NT)]
            ht1 = [hpool.tile([128, 128], F16, tag="ht1", bufs=NT,
                              name=f"ht1_{g}") for g in range(NT)]
            # tile 31's pad column (c=7) is read by the projection
            nc.vector.memset(ht0[NT - 1][:], 0.0)
            nc.vector.memset(ht1[NT - 1][:], 0.0)

            def hv(ht_g, r0, r1, c):
                # [r1-r0, 16] column view of step slot c (stride 8, offset c)
                return ht_g[r0:r1].rearrange("p (b t) -> p b t", t=8)[:, :, c]

            # ---------------- phase A: gather -> embT -> U in PSUM ----------
            bankA = {}
            bankB = {}
            gth_tiles = {}
            embt_tiles = {}

            def issue_gather(g):
                gth = wpool.tile([128, E], F32, tag="gather", bufs=6,
                                 name=f"gth_{g}")
                nc.gpsimd.indirect_dma_start(
                    out=gth[:], out_offset=None, in_=d_emb[:],
                    in_offset=bass.IndirectOffsetOnAxis(ap=ids_sb[:, g:g + 1], axis=0),
                )
                gth_tiles[g] = gth

            def prep_embt(g):
                # cast to fp16 (Pool), transpose on the PE, copy out on DVE
                gth = gth_tiles.pop(g)
                g16 = wpool.tile([128, E], F16, tag="g16", name=f"g16_{g}")
                nc.gpsimd.tensor_copy(g16[:], gth[:])
                embt = []
                for k in range(2):
                    tp = psum.tile([128, 128], F16, tag="tp", bufs=2, space="PSUM",
                                   name=f"tp_{g}_{k}")
                    nc.tensor.transpose(
                        out=tp[:], in_=g16[:, k * 128:(k + 1) * 128],
                        identity=ident16[:])
                    et = wpool.tile([128, 128], F16, tag=f"embt{k}", bufs=3,
                                    name=f"et_{g}_{k}")
                    nc.vector.tensor_copy(et[:], tp[:])
                    embt.append(et)
                embt_tiles[g] = embt

            def u_mms(g):
                embt = embt_tiles.pop(g)
                # U for blocks 0,1: all 128 cols (col = c*16 + b, t-major)
                ba = psum.tile([128, 128], F32, tag="bankA", bufs=2,
                               space="PSUM", name=f"bankA_{g}")
                for k in range(2):
                    nc.tensor.matmul(out=ba[:], lhsT=wiA[k][:], rhs=embt[k][:],
                                     start=(k == 0), stop=(k == 1))
                bankA[g] = ba
                # U for blocks 3,2 (rows 0:64 = block3, 64:128 = block2) at
                # step slots c=0 / c=4; single start/stop pair per bank
                # (start marks the whole 2KB zero region pending-zero)
                bb = psum.tile([128, 32], F32, tag="bankB", bufs=2,
                               space="PSUM", name=f"bankB_{g}")
                for k in range(2):
                    src = embt[k][:].rearrange("p (c2 r) -> p c2 r", c2=2)
                    nc.tensor.matmul(out=bb[:, 0:32], lhsT=wiB[k][:],
                                     rhs=src[:, :, 0:16],
                                     start=(k == 0), stop=(k == 1))
                bankB[g] = bb

            for g0 in range(4):
                issue_gather(g0)
            prep_embt(0)
            prep_embt(1)
            u_mms(0)

            # ---------------- projection pacing ----------------
            from collections import deque
            proj_q = deque()   # pending (g, unit) items; unit = (p, k, vc)
            ob_tiles = {}
            done_chunks = {}

            def enqueue_proj(g):
                # 8 units of 2 matmuls each; k0 units start a pair of PSUM
                # banks, k1 units finish + drain them
                for p in range(2):
                    for pair in range(2):
                        for k in range(2):
                            proj_q.append((g, p, pair, k))

            pp_banks = {}

            def emit_proj_unit():
                if not proj_q:
                    return
                g, p, pair, k = proj_q.popleft()
                if g not in ob_tiles:
                    ob_tiles[g] = opool.tile([128, VS], F16, tag="ob",
                                             name=f"ob_{g}")
                    done_chunks[g] = 0
                ht_g = ht0[g] if k == 0 else ht1[g]
                drains = []
                for vc in (2 * pair, 2 * pair + 1):
                    col = p * 2000 + vc * VC
                    if k == 0:
                        pp = psum.tile([128, VC], F32, tag="pp", bufs=2,
                                       space="PSUM", name=f"pp_{g}_{p}_{vc}")
                        pp_banks[(g, p, vc)] = pp
                        nc.tensor.matmul(out=pp[:], lhsT=ht_g[:],
                                         rhs=wo16[0][:, col:col + VC],
                                         start=True, stop=False)
                    else:
                        pp = pp_banks.pop((g, p, vc))
                        nc.tensor.matmul(out=pp[:], lhsT=ht_g[:],
                                         rhs=wo16[1][:, col:col + VC],
                                         start=False, stop=True)
                        drains.append((col, pp))
                for col, pp in drains:
                    # all drains on DVE: ACT must stay clear for the chain
                    # tanhs, gpsimd has no PSUM access
                    nc.vector.tensor_copy(ob_tiles[g][:, col:col + VC], pp[:])
                done_chunks[g] += len(drains)
                if done_chunks[g] == 8:
                    ob = ob_tiles.pop(g)
                    nc.sync.dma_start(out=d_out[g * 128:(g + 1) * 128, :],
                                      in_=ob[:])

            # ---------------- serial chains ----------------
            # per-step emission; chain1/2/3 are slotted to lag chain0.
            for t in range(T):
                g, c = divmod(t, 8)

                if c == 0 and g + 4 < NT:
                    issue_gather(g + 4)

                # --- chain0 (block0, every step) ---
                dst0 = hv(ht0[g], 0, 64, c)
                if t == 0:
                    nc.scalar.activation(dst0, bankA[0][0:64, 0:16], TANH,
                                         bias=biasA[0:64])
                else:
                    src = hv(ht0[g - 1], 0, 64, 7) if c == 0 else \
                        hv(ht0[g], 0, 64, c - 1)
                    nc.tensor.matmul(out=bankA[g][0:64, c * 16:(c + 1) * 16],
                                     lhsT=whA[0:64, 0:64], rhs=src,
                                     start=False, stop=True,
                                     skip_group_check=True)
                    nc.scalar.activation(dst0, bankA[g][0:64, c * 16:(c + 1) * 16],
                                         TANH, bias=biasA[0:64])

                # --- chain1 (block1, even steps) ---
                if c % 2 == 0:
                    dst1 = hv(ht0[g], 64, 128, c)
                    cc = slice(c * 16, (c + 1) * 16)
                    if t == 0:
                        nc.scalar.activation(dst1, bankA[0][64:128, 0:16], TANH,
                                             bias=biasA[64:128])
                    else:
                        self_src = hv(ht0[g], 64, 128, c - 2) if c >= 2 else \
                            hv(ht0[g - 1], 64, 128, 6)
                        nc.tensor.matmul(out=bankA[g][64:128, cc],
                                         lhsT=whA[64:128, 64:128], rhs=self_src,
                                         start=False, stop=True,
                                         skip_group_check=True)
                        nc.scalar.activation(dst1, bankA[g][64:128, cc], TANH,
                                             bias=biasA[64:128])
                    # held value for the odd step c+1 (off critical path)
                    v1 = ht0[g][64:128].rearrange("p (b t) -> p b t", t=8)
                    nc.gpsimd.tensor_copy(v1[:, :, c + 1], dst1)

                # --- cross block0 -> block1 for col c+1 (odd c) ---
                if c in (1, 3, 5) and t + 1 < T:
                    cc1 = slice((c + 1) * 16, (c + 2) * 16)
                    nc.tensor.matmul(out=bankA[g][64:128, cc1],
                                     lhsT=whA[0:64, 64:128],
                                     rhs=hv(ht0[g], 0, 64, c),
                                     start=False, stop=True,
                                     skip_group_check=True)
                if c == 7 and g + 1 < NT:
                    # cross into next tile's col 0
                    nc.tensor.matmul(out=bankA[g + 1][64:128, 0:16],
                                     lhsT=whA[0:64, 64:128],
                                     rhs=hv(ht0[g], 0, 64, 7),
                                     start=False, stop=True,
                                     skip_group_check=True)

                # --- chain3 (block3, t%8==0), slotted at c==1 ---
                if c == 1:
                    dst3 = hv(ht1[g], 0, 64, 0)
                    if g == 0:
                        nc.scalar.activation(dst3, bankB[0][0:64, 0:16], TANH,
                                             bias=biasB[0:64])
                    else:
                        # cross from blocks 0,1 at t-1
                        nc.tensor.matmul(out=bankB[g][0:64, 0:16],
                                         lhsT=wX3[:],
                                         rhs=ht0[g - 1][:].rearrange(
                                             "p (b t) -> p b t", t=8)[:, :, 7],
                                         start=False, stop=True,
                                         skip_group_check=True)
                        # self W33 + W23 (block2 state held at col 4)
                        nc.tensor.matmul(out=bankB[g][0:64, 0:16],
                                         lhsT=wB3[:],
                                         rhs=ht1[g - 1][:].rearrange(
                                             "p (b t) -> p b t", t=8)[:, :, 4],
                                         start=False, stop=True,
                                         skip_group_check=True)
                        nc.scalar.activation(dst3, bankB[g][0:64, 0:16], TANH,
                                             bias=biasB[0:64])
                    v3 = ht1[g][0:64].rearrange("p (b t) -> p b t", t=8)
                    nc.gpsimd.tensor_copy(
                        v3[:, :, 1:8],
                        dst3[:, :, None].to_broadcast([64, B, 7]))

                if c == 1 and g + 2 < NT:
                    prep_embt(g + 2)
                if c == 5 and g + 1 < NT:
                    u_mms(g + 1)

                # --- chain2 (block2, t%4==0), slotted at c==2 and c==5 ---
                if c == 2 or c == 5:
                    cs = 0 if c == 2 else 4          # step slot being computed
                    bb_cols = slice(0, 16) if cs == 0 else slice(16, 32)
                    dst2 = hv(ht1[g], 64, 128, cs)
                    if t <= 2:
                        nc.scalar.activation(dst2, bankB[0][64:128, bb_cols],
                                             TANH, bias=biasB[64:128])
                    else:
                        # cross from blocks 0,1 at t-1
                        xsrc = ht0[g - 1][:].rearrange(
                            "p (b t) -> p b t", t=8)[:, :, 7] if cs == 0 else \
                            ht0[g][:].rearrange(
                                "p (b t) -> p b t", t=8)[:, :, 3]
                        nc.tensor.matmul(out=bankB[g][64:128, bb_cols],
                                         lhsT=wX2[:], rhs=xsrc,
                                         start=False, stop=True,
                                         skip_group_check=True)
                        self_src = hv(ht1[g - 1], 64, 128, 4) if cs == 0 else \
                            hv(ht1[g], 64, 128, 0)
                        nc.tensor.matmul(out=bankB[g][64:128, bb_cols],
                                         lhsT=wB2[64:128, :], rhs=self_src,
                                         start=False, stop=True,
                                         skip_group_check=True)
                        nc.scalar.activation(dst2, bankB[g][64:128, bb_cols],
                                             TANH, bias=biasB[64:128])
                    span = 3 if cs == 0 else min(3, T - t + 1)
                    v2 = ht1[g][64:128].rearrange("p (b t) -> p b t", t=8)
                    nc.gpsimd.tensor_copy(
                        v2[:, :, cs + 1:cs + 1 + span],
                        dst2[:, :, None].to_broadcast([64, B, span]))

                # --- projection pacing: 1 unit (2 matmuls) per step ---
                if c == 7 and g >= PROJ_LAG:
                    enqueue_proj(g - PROJ_LAG)
                emit_proj_unit()

            # flush remaining projection work (tiles whose c==7 enqueue
            # never fired: the last PROJ_LAG tiles plus tile NT-1 itself)
            for g in range(NT - PROJ_LAG - 1, NT):
                enqueue_proj(g)
            while proj_q:
                emit_proj_unit()

            if DEBUG_HT:
                for g in range(NT):
                    # dbg row = g*128 + unit_partition, col = token slot b*8+c
                    nc.sync.dma_start(out=d_dbg[g * 128:(g + 1) * 128, 0:128],
                                      in_=ht0[g][:])
                    nc.sync.dma_start(out=d_dbg[g * 128:(g + 1) * 128, 128:256],
                                      in_=ht1[g][:])

    nc.finalize()
    return nc


_NC_CACHE = None
TRACE = False        # set by test harness to capture an NTFF profile
TRACE_KW = {}
LAST_RESULT = None   # BassKernelResults of the most recent run
DEBUG_HT = False     # add a debug output with the recorded h history


def kernel(x, x_sl, embedding, Wi, Wh, bi, bh, Wo):
    global _NC_CACHE, LAST_RESULT
    if _NC_CACHE is None:
        _NC_CACHE = build_program()
    nc = _NC_CACHE

    x = np.asarray(x)
    ids = np.ascontiguousarray(x[:, :T].T).reshape(-1)  # n = t*B + b
    ids_pad = np.zeros(128 * NT, np.int32)
    ids_pad[:B * T] = ids
    ids_dev = np.ascontiguousarray(ids_pad.reshape(NT, 128).T)

    embedding = np.ascontiguousarray(np.asarray(embedding, np.float32))
    Wh16 = np.asarray(Wh, np.float16)
    Wi16 = np.asarray(Wi, np.float16)
    biasv = (np.asarray(bi, np.float32) + np.asarray(bh, np.float32))
    Wo16 = np.asarray(Wo, np.float16)

    whA_h = np.ascontiguousarray(Wh16[0:128, 0:128])
    wB2_h = np.zeros((128, 64), np.float16)
    wB2_h[64:128] = Wh16[128:192, 128:192]
    wB3_h = np.zeros((128, 64), np.float16)
    wB3_h[0:64] = Wh16[192:256, 192:256]
    wB3_h[64:128] = Wh16[128:192, 192:256]
    wX2_h = np.ascontiguousarray(Wh16[0:128, 128:192])
    wX3_h = np.ascontiguousarray(Wh16[0:128, 192:256])
    wiA_h = np.ascontiguousarray(Wi16[:, 0:128])
    wiB_h = np.ascontiguousarray(
        np.concatenate([Wi16[:, 192:256], Wi16[:, 128:192]], axis=1))
    biasA_h = np.ascontiguousarray(biasv[0:128].reshape(128, 1))
    biasB_h = np.ascontiguousarray(
        np.concatenate([biasv[192:256], biasv[128:192]]).reshape(128, 1))

    in_maps = []
    for cidx in range(NCORES):
        sl = slice(cidx * VS, (cidx + 1) * VS)
        in_maps.append({
            "tok_ids": ids_dev,
            "embedding": embedding,
            "whA": whA_h, "wB2": wB2_h, "wB3": wB3_h,
            "wX2": wX2_h, "wX3": wX3_h,
            "wiA": wiA_h, "wiB": wiB_h,
            "biasA": biasA_h, "biasB": biasB_h,
            "wo0": np.ascontiguousarray(Wo16[0:128, sl]),
            "wo1": np.ascontiguousarray(
                np.concatenate([Wo16[192:256, sl], Wo16[128:192, sl]], axis=0)),
        })

    res = run_bass_kernel_spmd(nc, in_maps, core_ids=list(range(NCORES)),
                               trace=TRACE, **TRACE_KW)
    LAST_RESULT = res
    raw = np.concatenate([r["out"] for r in res.results], axis=1)  # [4096, V]
    out = raw.reshape(NT, B, 8, V).transpose(1, 0, 2, 3).reshape(B, NT * 8, V)
    return out[:, :T].astype(np.float32)


# revision 24
# speedup vs baseline: 1.3373x; 1.0302x over previous
"""CWRNN language-model kernel for 8 Trainium2 NeuronCores.

Strategy (vocab-sharded output projection, v2):
  - Each core owns Wo[:, c*4000:(c+1)*4000] and writes its logits slice in
    fp16 (tolerance 2e-2 >> fp16 rounding) -> halves the HBM write volume
    that dominated v1.
  - The clockwork mask is block-triangular: block0 (period 1, units 0:64)
    is fed only by itself, so the serial critical path is a 64-unit RNN.
    Blocks 1-3 run as separate serial chains lagging block0, with their
    cross-block input terms batched as per-tile matmuls over the recorded
    history tiles.
  - U = emb @ Wi is accumulated directly into PSUM banks (phase A); chain
    matmuls accumulate h @ Whh on top (start=False) and tanh reads PSUM
    with the bias folded into the activation -> each chain step is exactly
    matmul -> tanh, and tanh writes straight into the fp16 history tile
    column the next step's matmul reads (no DVE add, no Pool copy on the
    critical path).
  - Projection pairs two 500-col chunks per weight load, paced one unit
    per chain step two tiles behind the recurrence; all PSUM drains stay
    on DVE (ACT is reserved for the serial tanh chain; gpsimd has no PSUM
    access); one fp16 output DMA per tile on the sync queue.
  - Pacing note: the PE sustains only ~50% of its 2.4GHz peak (power
    throttle).  Denser projection pacing (more PSUM banks, bigger bursts)
    measurably LOWERS throughput once the throttle engages, so the
    drain-coupled pp=2 pacing is deliberate.
"""

import sys

sys.path.insert(0, "/opt/trn_rl_repo")

import numpy as np

import concourse.bass as bass
import concourse.mybir as mybir
import concourse.tile as tile
from concourse import bacc
from concourse import bass_utils as _bass_utils
from concourse.bass_utils import run_bass_kernel_spmd
from concourse.masks import make_identity

# note: walrus's --enable-ldw-opt pass crashes codegen on this toolchain
# (visitInstLdweights), so per-matmul LDWEIGHTS reloads are unavoidable

F32 = mybir.dt.float32
F16 = mybir.dt.float16
I32 = mybir.dt.int32
TANH = mybir.ActivationFunctionType.Tanh

B = 16
T = 255           # x[:, :-1]
E = 256
NH = 256
V = 32000
NCORES = 8
VS = V // NCORES  # 4000 vocab columns per core
NT = 32           # token tiles of 8 steps (tile 31 has 7 real steps)
VC = 500          # vocab chunk per PSUM bank
PROJ_LAG = 2      # tiles between recurrence and projection start


def build_program():
    nc = bacc.Bacc(target_bir_lowering=False)

    d_ids = nc.dram_tensor("tok_ids", [128, NT], I32, kind="ExternalInput")
    d_emb = nc.dram_tensor("embedding", [V + 1, E], F32, kind="ExternalInput")
    d_whA = nc.dram_tensor("whA", [128, 128], F16, kind="ExternalInput")
    d_wB2 = nc.dram_tensor("wB2", [128, 64], F16, kind="ExternalInput")
    d_wB3 = nc.dram_tensor("wB3", [128, 64], F16, kind="ExternalInput")
    d_wX2 = nc.dram_tensor("wX2", [128, 64], F16, kind="ExternalInput")
    d_wX3 = nc.dram_tensor("wX3", [128, 64], F16, kind="ExternalInput")
    d_wiA = nc.dram_tensor("wiA", [256, 128], F16, kind="ExternalInput")
    d_wiB = nc.dram_tensor("wiB", [256, 128], F16, kind="ExternalInput")
    d_biasA = nc.dram_tensor("biasA", [128, 1], F32, kind="ExternalInput")
    d_biasB = nc.dram_tensor("biasB", [128, 1], F32, kind="ExternalInput")
    d_wo0 = nc.dram_tensor("wo0", [128, VS], F16, kind="ExternalInput")
    d_wo1 = nc.dram_tensor("wo1", [128, VS], F16, kind="ExternalInput")
    # raw token-major output: row g*128 + b*8 + c  <->  logits[b, g*8+c]
    d_out = nc.dram_tensor("out", [NT * 128, VS], F16, kind="ExternalOutput")
    d_dbg = None
    if DEBUG_HT:
        d_dbg = nc.dram_tensor("dbg_ht", [NT * 128, 256], F16,
                               kind="ExternalOutput")

    with tile.TileContext(nc) as tc:
        with tc.tile_pool(name="const", bufs=1) as cpool, \
             tc.tile_pool(name="hist", bufs=1) as hpool, \
             tc.tile_pool(name="obuf", bufs=3) as opool, \
             tc.tile_pool(name="work", bufs=3) as wpool, \
             tc.tile_pool(name="psum", bufs=2, space="PSUM") as psum:

            # ---------------- constants and weights ----------------
            ids_sb = cpool.tile([128, NT], I32)
            nc.sync.dma_start(out=ids_sb[:], in_=d_ids[:])

            whA = cpool.tile([128, 128], F16, name="whA")
            nc.sync.dma_start(out=whA[:], in_=d_whA[:])
            wB2 = cpool.tile([128, 64], F16, name="wB2")
            nc.sync.dma_start(out=wB2[:], in_=d_wB2[:])
            wB3 = cpool.tile([128, 64], F16, name="wB3")
            nc.sync.dma_start(out=wB3[:], in_=d_wB3[:])
            wX2 = cpool.tile([128, 64], F16, name="wX2")
            nc.sync.dma_start(out=wX2[:], in_=d_wX2[:])
            wX3 = cpool.tile([128, 64], F16, name="wX3")
            nc.sync.dma_start(out=wX3[:], in_=d_wX3[:])
            wiA = [cpool.tile([128, 128], F16, name=f"wiA{k}") for k in range(2)]
            wiB = [cpool.tile([128, 128], F16, name=f"wiB{k}") for k in range(2)]
            for k in range(2):
                nc.sync.dma_start(out=wiA[k][:], in_=d_wiA[k * 128:(k + 1) * 128, :])
                nc.sync.dma_start(out=wiB[k][:], in_=d_wiB[k * 128:(k + 1) * 128, :])
            biasA = cpool.tile([128, 1], F32, name="biasA")
            nc.sync.dma_start(out=biasA[:], in_=d_biasA[:])
            biasB = cpool.tile([128, 1], F32, name="biasB")
            nc.sync.dma_start(out=biasB[:], in_=d_biasB[:])

            ident16 = cpool.tile([128, 128], F16)
            make_identity(nc, ident16[:])

            wo16 = []
            for k, d_wo in enumerate((d_wo0, d_wo1)):
                wo = cpool.tile([128, VS], F16, name=f"wo16_{k}")
                nc.scalar.dma_start(out=wo[:], in_=d_wo[:])
                wo16.append(wo)

            # fp16 history tiles, one per token tile; col = b*8 + c
            ht0 = [hpool.tile([128, 128], F16, tag="ht0", bufs=NT,
                              name=f"ht0_{g}") for g in range(NT)]
            ht1 = [hpool.tile([128, 128], F16, tag="ht1", bufs=NT,
                              name=f"ht1_{g}") for g in range(NT)]
            # tile 31's pad column (c=7) is read by the projection
            nc.vector.memset(ht0[NT - 1][:], 0.0)
            nc.vector.memset(ht1[NT - 1][:], 0.0)

            def hv(ht_g, r0, r1, c):
                # [r1-r0, 16] column view of step slot c (stride 8, offset c)
                return ht_g[r0:r1].rearrange("p (b t) -> p b t", t=8)[:, :, c]

            # ---------------- phase A: gather -> embT -> U in PSUM ----------
            bankA = {}
            bankB = {}
            gth_tiles = {}
            embt_tiles = {}

            def issue_gather(g):
                gth = wpool.tile([128, E], F32, tag="gather", bufs=6,
                                 name=f"gth_{g}")
                nc.gpsimd.indirect_dma_start(
                    out=gth[:], out_offset=None, in_=d_emb[:],
                    in_offset=bass.IndirectOffsetOnAxis(ap=ids_sb[:, g:g + 1], axis=0),
                )
                gth_tiles[g] = gth

            def prep_embt(g):
                # cast to fp16 (Pool), transpose on the PE, copy out on DVE
                gth = gth_tiles.pop(g)
                g16 = wpool.tile([128, E], F16, tag="g16", name=f"g16_{g}")
                nc.gpsimd.tensor_copy(g16[:], gth[:])
                embt = []
                for k in range(2):
                    tp = psum.tile([128, 128], F16, tag="tp", bufs=2, space="PSUM",
                                   name=f"tp_{g}_{k}")
                    nc.tensor.transpose(
                        out=tp[:], in_=g16[:, k * 128:(k + 1) * 128],
                        identity=ident16[:])
                    et = wpool.tile([128, 128], F16, tag=f"embt{k}", bufs=3,
                                    name=f"et_{g}_{k}")
                    nc.vector.tensor_copy(et[:], tp[:])
                    embt.append(et)
                embt_tiles[g] = embt

            def u_mms(g):
                embt = embt_tiles.pop(g)
                # U for blocks 0,1: all 128 cols (col = c*16 + b, t-major)
                ba = psum.tile([128, 128], F32, tag="bankA", bufs=2,
                               space="PSUM", name=f"bankA_{g}")
                for k in range(2):
                    nc.tensor.matmul(out=ba[:], lhsT=wiA[k][:], rhs=embt[k][:],
                                     start=(k == 0), stop=(k == 1))
                bankA[g] = ba
                # U for blocks 3,2 (rows 0:64 = block3, 64:128 = block2) at
                # step slots c=0 / c=4; single start/stop pair per bank
                # (start marks the whole 2KB zero region pending-zero)
                bb = psum.tile([128, 32], F32, tag="bankB", bufs=2,
                               space="PSUM", name=f"bankB_{g}")
                for k in range(2):
                    src = embt[k][:].rearrange("p (c2 r) -> p c2 r", c2=2)
                    nc.tensor.matmul(out=bb[:, 0:32], lhsT=wiB[k][:],
                                     rhs=src[:, :, 0:16],
                                     start=(k == 0), stop=(k == 1))
                bankB[g] = bb

            for g0 in range(4):
                issue_gather(g0)
            prep_embt(0)
            prep_embt(1)
            u_mms(0)

            # ---------------- projection pacing ----------------
            from collections import deque
            proj_q = deque()   # pending (g, unit) items; unit = (p, k, vc)
            ob_tiles = {}
            done_chunks = {}

            def enqueue_proj(g):
                # 8 units of 2 matmuls each; k0 units start a pair of PSUM
                # banks, k1 units finish + drain them
                for p in range(2):
                    for pair in range(2):
                        for k in range(2):
                            proj_q.append((g, p, pair, k))

            pp_banks = {}

            def emit_proj_unit():
                if not proj_q:
                    return
                g, p, pair, k = proj_q.popleft()
                if g not in ob_tiles:
                    ob_tiles[g] = opool.tile([128, VS], F16, tag="ob",
                                             name=f"ob_{g}")
                    done_chunks[g] = 0
                ht_g = ht0[g] if k == 0 else ht1[g]
                drains = []
                for vc in (2 * pair, 2 * pair + 1):
                    col = p * 2000 + vc * VC
                    if k == 0:
                        pp = psum.tile([128, VC], F32, tag="pp", bufs=2,
                                       space="PSUM", name=f"pp_{g}_{p}_{vc}")
                        pp_banks[(g, p, vc)] = pp
                        nc.tensor.matmul(out=pp[:], lhsT=ht_g[:],
                                         rhs=wo16[0][:, col:col + VC],
                                         start=True, stop=False)
                    else:
                        pp = pp_banks.pop((g, p, vc))
                        nc.tensor.matmul(out=pp[:], lhsT=ht_g[:],
                                         rhs=wo16[1][:, col:col + VC],
                                         start=False, stop=True)
                        drains.append((col, pp))
                for col, pp in drains:
                    # all drains on DVE: ACT must stay clear for the chain
                    # tanhs, gpsimd has no PSUM access
                    nc.vector.tensor_copy(ob_tiles[g][:, col:col + VC], pp[:])
                done_chunks[g] += len(drains)
                if done_chunks[g] == 8:
                    ob = ob_tiles.pop(g)
                    nc.sync.dma_start(out=d_out[g * 128:(g + 1) * 128, :],
                                      in_=ob[:])

            # ---------------- serial chains ----------------
            # per-step emission; chain1/2/3 are slotted to lag chain0.
            for t in range(T):
                g, c = divmod(t, 8)

                if c == 0 and g + 4 < NT:
                    issue_gather(g + 4)

                # --- chain0 (block0, every step) ---
                # At even steps the W00 and W01 products share rhs =
                # h0_{t-1}, so one matmul with lhsT = [W00|W01] computes
                # both the chain0 input (rows 0:64) and block1's cross
                # term (rows 64:128) -> no separate cross matmuls.
                dst0 = hv(ht0[g], 0, 64, c)
                if t == 0:
                    nc.scalar.activation(dst0, bankA[0][0:64, 0:16], TANH,
                                         bias=biasA[0:64])
                else:
                    src = hv(ht0[g - 1], 0, 64, 7) if c == 0 else \
                        hv(ht0[g], 0, 64, c - 1)
                    rows = slice(0, 128) if c % 2 == 0 else slice(0, 64)
                    nc.tensor.matmul(out=bankA[g][rows, c * 16:(c + 1) * 16],
                                     lhsT=whA[0:64, rows], rhs=src,
                                     start=False, stop=True,
                                     skip_group_check=True)
                    nc.scalar.activation(dst0, bankA[g][0:64, c * 16:(c + 1) * 16],
                                         TANH, bias=biasA[0:64])

                # --- chain1 (block1, even steps) ---
                if c % 2 == 0:
                    dst1 = hv(ht0[g], 64, 128, c)
                    cc = slice(c * 16, (c + 1) * 16)
                    if t == 0:
                        nc.scalar.activation(dst1, bankA[0][64:128, 0:16], TANH,
                                             bias=biasA[64:128])
                    else:
                        self_src = hv(ht0[g], 64, 128, c - 2) if c >= 2 else \
                            hv(ht0[g - 1], 64, 128, 6)
                        nc.tensor.matmul(out=bankA[g][64:128, cc],
                                         lhsT=whA[64:128, 64:128], rhs=self_src,
                                         start=False, stop=True,
                                         skip_group_check=True)
                        nc.scalar.activation(dst1, bankA[g][64:128, cc], TANH,
                                             bias=biasA[64:128])
                    # held value for the odd step c+1 (off critical path)
                    v1 = ht0[g][64:128].rearrange("p (b t) -> p b t", t=8)
                    nc.gpsimd.tensor_copy(v1[:, :, c + 1], dst1)

                # --- chain3 (block3, t%8==0), slotted at c==1 ---
                if c == 1:
                    dst3 = hv(ht1[g], 0, 64, 0)
                    if g == 0:
                        nc.scalar.activation(dst3, bankB[0][0:64, 0:16], TANH,
                                             bias=biasB[0:64])
                    else:
                        # cross from blocks 0,1 at t-1
                        nc.tensor.matmul(out=bankB[g][0:64, 0:16],
                                         lhsT=wX3[:],
                                         rhs=ht0[g - 1][:].rearrange(
                                             "p (b t) -> p b t", t=8)[:, :, 7],
                                         start=False, stop=True,
                                         skip_group_check=True)
                        # self W33 + W23 (block2 state held at col 4)
                        nc.tensor.matmul(out=bankB[g][0:64, 0:16],
                                         lhsT=wB3[:],
                                         rhs=ht1[g - 1][:].rearrange(
                                             "p (b t) -> p b t", t=8)[:, :, 4],
                                         start=False, stop=True,
                                         skip_group_check=True)
                        nc.scalar.activation(dst3, bankB[g][0:64, 0:16], TANH,
                                             bias=biasB[0:64])
                    v3 = ht1[g][0:64].rearrange("p (b t) -> p b t", t=8)
                    nc.gpsimd.tensor_copy(
                        v3[:, :, 1:8],
                        dst3[:, :, None].to_broadcast([64, B, 7]))

                if c == 1 and g + 2 < NT:
                    prep_embt(g + 2)
                if c == 5 and g + 1 < NT:
                    u_mms(g + 1)

                # --- chain2 (block2, t%4==0), slotted at c==2 and c==5 ---
                if c == 2 or c == 5:
                    cs = 0 if c == 2 else 4          # step slot being computed
                    bb_cols = slice(0, 16) if cs == 0 else slice(16, 32)
                    dst2 = hv(ht1[g], 64, 128, cs)
                    if t <= 2:
                        nc.scalar.activation(dst2, bankB[0][64:128, bb_cols],
                                             TANH, bias=biasB[64:128])
                    else:
                        # cross from blocks 0,1 at t-1
                        xsrc = ht0[g - 1][:].rearrange(
                            "p (b t) -> p b t", t=8)[:, :, 7] if cs == 0 else \
                            ht0[g][:].rearrange(
                                "p (b t) -> p b t", t=8)[:, :, 3]
                        nc.tensor.matmul(out=bankB[g][64:128, bb_cols],
                                         lhsT=wX2[:], rhs=xsrc,
                                         start=False, stop=True,
                                         skip_group_check=True)
                        self_src = hv(ht1[g - 1], 64, 128, 4) if cs == 0 else \
                            hv(ht1[g], 64, 128, 0)
                        nc.tensor.matmul(out=bankB[g][64:128, bb_cols],
                                         lhsT=wB2[64:128, :], rhs=self_src,
                                         start=False, stop=True,
                                         skip_group_check=True)
                        nc.scalar.activation(dst2, bankB[g][64:128, bb_cols],
                                             TANH, bias=biasB[64:128])
                    span = 3 if cs == 0 else min(3, T - t + 1)
                    v2 = ht1[g][64:128].rearrange("p (b t) -> p b t", t=8)
                    nc.gpsimd.tensor_copy(
                        v2[:, :, cs + 1:cs + 1 + span],
                        dst2[:, :, None].to_broadcast([64, B, span]))

                # --- projection pacing: 1 unit (2 matmuls) per step ---
                if c == 7 and g >= PROJ_LAG:
                    enqueue_proj(g - PROJ_LAG)
                emit_proj_unit()

            # flush remaining projection work (tiles whose c==7 enqueue
            # never fired: the last PROJ_LAG tiles plus tile NT-1 itself)
            for g in range(NT - PROJ_LAG - 1, NT):
                enqueue_proj(g)
            while proj_q:
                emit_proj_unit()

            if DEBUG_HT:
                for g in range(NT):
                    # dbg row = g*128 + unit_partition, col = token slot b*8+c
                    nc.sync.dma_start(out=d_dbg[g * 128:(g + 1) * 128, 0:128],
                                      in_=ht0[g][:])
                    nc.sync.dma_start(out=d_dbg[g * 128:(g + 1) * 128, 128:256],
                                      in_=ht1[g][:])

    nc.finalize()
    return nc


_NC_CACHE = None
TRACE = False        # set by test harness to capture an NTFF profile
TRACE_KW = {}
LAST_RESULT = None   # BassKernelResults of the most recent run
DEBUG_HT = False     # add a debug output with the recorded h history


def kernel(x, x_sl, embedding, Wi, Wh, bi, bh, Wo):
    global _NC_CACHE, LAST_RESULT
    if _NC_CACHE is None:
        _NC_CACHE = build_program()
    nc = _NC_CACHE

    x = np.asarray(x)
    ids = np.ascontiguousarray(x[:, :T].T).reshape(-1)  # n = t*B + b
    ids_pad = np.zeros(128 * NT, np.int32)
    ids_pad[:B * T] = ids
    ids_dev = np.ascontiguousarray(ids_pad.reshape(NT, 128).T)

    embedding = np.ascontiguousarray(np.asarray(embedding, np.float32))
    Wh16 = np.asarray(Wh, np.float16)
    Wi16 = np.asarray(Wi, np.float16)
    biasv = (np.asarray(bi, np.float32) + np.asarray(bh, np.float32))
    Wo16 = np.asarray(Wo, np.float16)

    whA_h = np.ascontiguousarray(Wh16[0:128, 0:128])
    wB2_h = np.zeros((128, 64), np.float16)
    wB2_h[64:128] = Wh16[128:192, 128:192]
    wB3_h = np.zeros((128, 64), np.float16)
    wB3_h[0:64] = Wh16[192:256, 192:256]
    wB3_h[64:128] = Wh16[128:192, 192:256]
    wX2_h = np.ascontiguousarray(Wh16[0:128, 128:192])
    wX3_h = np.ascontiguousarray(Wh16[0:128, 192:256])
    wiA_h = np.ascontiguousarray(Wi16[:, 0:128])
    wiB_h = np.ascontiguousarray(
        np.concatenate([Wi16[:, 192:256], Wi16[:, 128:192]], axis=1))
    biasA_h = np.ascontiguousarray(biasv[0:128].reshape(128, 1))
    biasB_h = np.ascontiguousarray(
        np.concatenate([biasv[192:256], biasv[128:192]]).reshape(128, 1))

    in_maps = []
    for cidx in range(NCORES):
        sl = slice(cidx * VS, (cidx + 1) * VS)
        in_maps.append({
            "tok_ids": ids_dev,
            "embedding": embedding,
            "whA": whA_h, "wB2": wB2_h, "wB3": wB3_h,
            "wX2": wX2_h, "wX3": wX3_h,
            "wiA": wiA_h, "wiB": wiB_h,
            "biasA": biasA_h, "biasB": biasB_h,
            "wo0": np.ascontiguousarray(Wo16[0:128, sl]),
            "wo1": np.ascontiguousarray(
                np.concatenate([Wo16[192:256, sl], Wo16[128:192, sl]], axis=0)),
        })

    res = run_bass_kernel_spmd(nc, in_maps, core_ids=list(range(NCORES)),
                               trace=TRACE, **TRACE_KW)
    LAST_RESULT = res
    raw = np.concatenate([r["out"] for r in res.results], axis=1)  # [4096, V]
    out = raw.reshape(NT, B, 8, V).transpose(1, 0, 2, 3).reshape(B, NT * 8, V)
    return out[:, :T].astype(np.float32)


# revision 25
# speedup vs baseline: 1.4543x; 1.0875x over previous
"""CWRNN language-model kernel for 8 Trainium2 NeuronCores.

Strategy (vocab-sharded output projection, v2):
  - Each core owns Wo[:, c*4000:(c+1)*4000] and writes its logits slice in
    fp16 (tolerance 2e-2 >> fp16 rounding) -> halves the HBM write volume
    that dominated v1.
  - The clockwork mask is block-triangular: block0 (period 1, units 0:64)
    is fed only by itself, so the serial critical path is a 64-unit RNN.
    Blocks 1-3 run as separate serial chains lagging block0, with their
    cross-block input terms batched as per-tile matmuls over the recorded
    history tiles.
  - U = emb @ Wi is accumulated directly into PSUM banks (phase A); chain
    matmuls accumulate h @ Whh on top (start=False) and tanh reads PSUM
    with the bias folded into the activation -> each chain step is exactly
    matmul -> tanh, and tanh writes straight into the fp16 history tile
    column the next step's matmul reads (no DVE add, no Pool copy on the
    critical path).
  - Projection pairs two 500-col chunks per weight load, paced one unit
    per chain step two tiles behind the recurrence; all PSUM drains stay
    on DVE (ACT is reserved for the serial tanh chain; gpsimd has no PSUM
    access); one fp16 output DMA per tile on the sync queue.
  - Pacing note: the PE sustains only ~50% of its 2.4GHz peak (power
    throttle).  Denser projection pacing (more PSUM banks, bigger bursts)
    measurably LOWERS throughput once the throttle engages, so the
    drain-coupled pp=2 pacing is deliberate.
"""

import sys

sys.path.insert(0, "/opt/trn_rl_repo")

import numpy as np

import concourse.bass as bass
import concourse.mybir as mybir
import concourse.tile as tile
from concourse import bacc
from concourse import bass_utils as _bass_utils
from concourse.bass_utils import run_bass_kernel_spmd
from concourse.masks import make_identity

# note: walrus's --enable-ldw-opt pass crashes codegen on this toolchain
# (visitInstLdweights), so per-matmul LDWEIGHTS reloads are unavoidable

F32 = mybir.dt.float32
F16 = mybir.dt.float16
I32 = mybir.dt.int32
TANH = mybir.ActivationFunctionType.Tanh

B = 16
T = 255           # x[:, :-1]
E = 256
NH = 256
V = 32000
NCORES = 8
VS = V // NCORES  # 4000 vocab columns per core
NT = 32           # token tiles of 8 steps (tile 31 has 7 real steps)
VC = 500          # vocab chunk per PSUM bank
PROJ_LAG = 2      # tiles between recurrence and projection start


def build_program():
    nc = bacc.Bacc(target_bir_lowering=False)

    d_ids = nc.dram_tensor("tok_ids", [128, NT], I32, kind="ExternalInput")
    d_emb = nc.dram_tensor("embedding", [V + 1, E], F32, kind="ExternalInput")
    d_whA = nc.dram_tensor("whA", [128, 128], F16, kind="ExternalInput")
    d_wB23 = nc.dram_tensor("wB23", [128, 128], F16, kind="ExternalInput")
    d_wX23 = nc.dram_tensor("wX23", [128, 128], F16, kind="ExternalInput")
    d_wiA = nc.dram_tensor("wiA", [256, 128], F16, kind="ExternalInput")
    d_wiB = nc.dram_tensor("wiB", [256, 128], F16, kind="ExternalInput")
    d_biasA = nc.dram_tensor("biasA", [128, 1], F32, kind="ExternalInput")
    d_biasB = nc.dram_tensor("biasB", [128, 1], F32, kind="ExternalInput")
    d_wo0 = nc.dram_tensor("wo0", [128, VS], F16, kind="ExternalInput")
    d_wo1 = nc.dram_tensor("wo1", [128, VS], F16, kind="ExternalInput")
    # raw token-major output: row g*128 + b*8 + c  <->  logits[b, g*8+c]
    d_out = nc.dram_tensor("out", [NT * 128, VS], F16, kind="ExternalOutput")
    d_dbg = None
    if DEBUG_HT:
        d_dbg = nc.dram_tensor("dbg_ht", [NT * 128, 256], F16,
                               kind="ExternalOutput")

    with tile.TileContext(nc) as tc:
        with tc.tile_pool(name="const", bufs=1) as cpool, \
             tc.tile_pool(name="hist", bufs=1) as hpool, \
             tc.tile_pool(name="obuf", bufs=3) as opool, \
             tc.tile_pool(name="work", bufs=3) as wpool, \
             tc.tile_pool(name="psum", bufs=2, space="PSUM") as psum:

            # ---------------- constants and weights ----------------
            ids_sb = cpool.tile([128, NT], I32)
            nc.sync.dma_start(out=ids_sb[:], in_=d_ids[:])

            whA = cpool.tile([128, 128], F16, name="whA")
            nc.sync.dma_start(out=whA[:], in_=d_whA[:])
            wB23 = cpool.tile([128, 128], F16, name="wB23")
            nc.sync.dma_start(out=wB23[:], in_=d_wB23[:])
            wX23 = cpool.tile([128, 128], F16, name="wX23")
            nc.sync.dma_start(out=wX23[:], in_=d_wX23[:])
            wiA = [cpool.tile([128, 128], F16, name=f"wiA{k}") for k in range(2)]
            wiB = [cpool.tile([128, 128], F16, name=f"wiB{k}") for k in range(2)]
            for k in range(2):
                nc.sync.dma_start(out=wiA[k][:], in_=d_wiA[k * 128:(k + 1) * 128, :])
                nc.sync.dma_start(out=wiB[k][:], in_=d_wiB[k * 128:(k + 1) * 128, :])
            biasA = cpool.tile([128, 1], F32, name="biasA")
            nc.sync.dma_start(out=biasA[:], in_=d_biasA[:])
            biasB = cpool.tile([128, 1], F32, name="biasB")
            nc.sync.dma_start(out=biasB[:], in_=d_biasB[:])

            ident16 = cpool.tile([128, 128], F16)
            make_identity(nc, ident16[:])

            wo16 = []
            for k, d_wo in enumerate((d_wo0, d_wo1)):
                wo = cpool.tile([128, VS], F16, name=f"wo16_{k}")
                nc.scalar.dma_start(out=wo[:], in_=d_wo[:])
                wo16.append(wo)

            # fp16 history tiles, one per token tile; col = b*8 + c
            ht0 = [hpool.tile([128, 128], F16, tag="ht0", bufs=NT,
                              name=f"ht0_{g}") for g in range(NT)]
            ht1 = [hpool.tile([128, 128], F16, tag="ht1", bufs=NT,
                              name=f"ht1_{g}") for g in range(NT)]
            # tile 31's pad column (c=7) is read by the projection
            nc.vector.memset(ht0[NT - 1][:], 0.0)
            nc.vector.memset(ht1[NT - 1][:], 0.0)

            def hv(ht_g, r0, r1, c):
                # [r1-r0, 16] column view of step slot c (stride 8, offset c)
                return ht_g[r0:r1].rearrange("p (b t) -> p b t", t=8)[:, :, c]

            # ---------------- phase A: gather -> embT -> U in PSUM ----------
            bankA = {}
            bankB = {}
            gth_tiles = {}
            embt_tiles = {}

            def issue_gather(g):
                gth = wpool.tile([128, E], F32, tag="gather", bufs=6,
                                 name=f"gth_{g}")
                nc.gpsimd.indirect_dma_start(
                    out=gth[:], out_offset=None, in_=d_emb[:],
                    in_offset=bass.IndirectOffsetOnAxis(ap=ids_sb[:, g:g + 1], axis=0),
                )
                gth_tiles[g] = gth

            def prep_embt(g):
                # cast to fp16 (Pool), transpose on the PE, copy out on DVE
                gth = gth_tiles.pop(g)
                g16 = wpool.tile([128, E], F16, tag="g16", name=f"g16_{g}")
                nc.gpsimd.tensor_copy(g16[:], gth[:])
                embt = []
                for k in range(2):
                    tp = psum.tile([128, 128], F16, tag="tp", bufs=2, space="PSUM",
                                   name=f"tp_{g}_{k}")
                    nc.tensor.transpose(
                        out=tp[:], in_=g16[:, k * 128:(k + 1) * 128],
                        identity=ident16[:])
                    et = wpool.tile([128, 128], F16, tag=f"embt{k}", bufs=3,
                                    name=f"et_{g}_{k}")
                    nc.vector.tensor_copy(et[:], tp[:])
                    embt.append(et)
                embt_tiles[g] = embt

            def u_mms(g):
                embt = embt_tiles.pop(g)
                # U for blocks 0,1: all 128 cols (col = c*16 + b, t-major)
                ba = psum.tile([128, 128], F32, tag="bankA", bufs=2,
                               space="PSUM", name=f"bankA_{g}")
                for k in range(2):
                    nc.tensor.matmul(out=ba[:], lhsT=wiA[k][:], rhs=embt[k][:],
                                     start=(k == 0), stop=(k == 1))
                bankA[g] = ba
                # U for blocks 3,2 (rows 0:64 = block3, 64:128 = block2) at
                # step slots c=0 / c=4; single start/stop pair per bank
                # (start marks the whole 2KB zero region pending-zero)
                bb = psum.tile([128, 32], F32, tag="bankB", bufs=2,
                               space="PSUM", name=f"bankB_{g}")
                for k in range(2):
                    src = embt[k][:].rearrange("p (c2 r) -> p c2 r", c2=2)
                    nc.tensor.matmul(out=bb[:, 0:32], lhsT=wiB[k][:],
                                     rhs=src[:, :, 0:16],
                                     start=(k == 0), stop=(k == 1))
                bankB[g] = bb

            for g0 in range(4):
                issue_gather(g0)
            prep_embt(0)
            prep_embt(1)
            u_mms(0)

            # ---------------- projection pacing ----------------
            from collections import deque
            proj_q = deque()   # pending (g, unit) items; unit = (p, k, vc)
            ob_tiles = {}
            done_chunks = {}

            def enqueue_proj(g):
                # 8 units of 2 matmuls each; k0 units start a pair of PSUM
                # banks, k1 units finish + drain them
                for p in range(2):
                    for pair in range(2):
                        for k in range(2):
                            proj_q.append((g, p, pair, k))

            pp_banks = {}

            def emit_proj_unit():
                if not proj_q:
                    return
                g, p, pair, k = proj_q.popleft()
                if g not in ob_tiles:
                    ob_tiles[g] = opool.tile([128, VS], F16, tag="ob",
                                             name=f"ob_{g}")
                    done_chunks[g] = 0
                ht_g = ht0[g] if k == 0 else ht1[g]
                drains = []
                for vc in (2 * pair, 2 * pair + 1):
                    col = p * 2000 + vc * VC
                    if k == 0:
                        pp = psum.tile([128, VC], F32, tag="pp", bufs=2,
                                       space="PSUM", name=f"pp_{g}_{p}_{vc}")
                        pp_banks[(g, p, vc)] = pp
                        nc.tensor.matmul(out=pp[:], lhsT=ht_g[:],
                                         rhs=wo16[0][:, col:col + VC],
                                         start=True, stop=False)
                    else:
                        pp = pp_banks.pop((g, p, vc))
                        nc.tensor.matmul(out=pp[:], lhsT=ht_g[:],
                                         rhs=wo16[1][:, col:col + VC],
                                         start=False, stop=True)
                        drains.append((col, pp))
                for col, pp in drains:
                    # all drains on DVE: ACT must stay clear for the chain
                    # tanhs, gpsimd has no PSUM access
                    nc.vector.tensor_copy(ob_tiles[g][:, col:col + VC], pp[:])
                done_chunks[g] += len(drains)
                if done_chunks[g] == 8:
                    ob = ob_tiles.pop(g)
                    nc.sync.dma_start(out=d_out[g * 128:(g + 1) * 128, :],
                                      in_=ob[:])

            # ---------------- serial chains ----------------
            # per-step emission; chain1/2/3 are slotted to lag chain0.
            for t in range(T):
                g, c = divmod(t, 8)

                if c == 0 and g + 4 < NT:
                    issue_gather(g + 4)

                # --- chain0 (block0, every step) ---
                # At even steps the W00 and W01 products share rhs =
                # h0_{t-1}, so one matmul with lhsT = [W00|W01] computes
                # both the chain0 input (rows 0:64) and block1's cross
                # term (rows 64:128) -> no separate cross matmuls.
                dst0 = hv(ht0[g], 0, 64, c)
                if t == 0:
                    nc.scalar.activation(dst0, bankA[0][0:64, 0:16], TANH,
                                         bias=biasA[0:64])
                else:
                    src = hv(ht0[g - 1], 0, 64, 7) if c == 0 else \
                        hv(ht0[g], 0, 64, c - 1)
                    rows = slice(0, 128) if c % 2 == 0 else slice(0, 64)
                    nc.tensor.matmul(out=bankA[g][rows, c * 16:(c + 1) * 16],
                                     lhsT=whA[0:64, rows], rhs=src,
                                     start=False, stop=True,
                                     skip_group_check=True)
                    nc.scalar.activation(dst0, bankA[g][0:64, c * 16:(c + 1) * 16],
                                         TANH, bias=biasA[0:64])

                # --- chain1 (block1, even steps) ---
                if c % 2 == 0:
                    dst1 = hv(ht0[g], 64, 128, c)
                    cc = slice(c * 16, (c + 1) * 16)
                    if t == 0:
                        nc.scalar.activation(dst1, bankA[0][64:128, 0:16], TANH,
                                             bias=biasA[64:128])
                    else:
                        self_src = hv(ht0[g], 64, 128, c - 2) if c >= 2 else \
                            hv(ht0[g - 1], 64, 128, 6)
                        nc.tensor.matmul(out=bankA[g][64:128, cc],
                                         lhsT=whA[64:128, 64:128], rhs=self_src,
                                         start=False, stop=True,
                                         skip_group_check=True)
                        nc.scalar.activation(dst1, bankA[g][64:128, cc], TANH,
                                             bias=biasA[64:128])
                    # held value for the odd step c+1 (off critical path)
                    v1 = ht0[g][64:128].rearrange("p (b t) -> p b t", t=8)
                    nc.gpsimd.tensor_copy(v1[:, :, c + 1], dst1)

                # --- chain3 + chain2 slot 0 (both update at t%8==0),
                # slotted at c==1; stacked weights -> 2 matmuls, not 4 ---
                if c == 1:
                    dst3 = hv(ht1[g], 0, 64, 0)
                    dst2a = hv(ht1[g], 64, 128, 0)
                    if g == 0:
                        nc.scalar.activation(dst3, bankB[0][0:64, 0:16], TANH,
                                             bias=biasB[0:64])
                        nc.scalar.activation(dst2a, bankB[0][64:128, 0:16],
                                             TANH, bias=biasB[64:128])
                    else:
                        # cross from blocks 0,1 at t-1 into [b3; b2]
                        nc.tensor.matmul(out=bankB[g][0:128, 0:16],
                                         lhsT=wX23[:],
                                         rhs=ht0[g - 1][:].rearrange(
                                             "p (b t) -> p b t", t=8)[:, :, 7],
                                         start=False, stop=True,
                                         skip_group_check=True)
                        # self terms [W33 h3 + W23 h2 ; W22 h2] (col 4 held)
                        nc.tensor.matmul(out=bankB[g][0:128, 0:16],
                                         lhsT=wB23[:],
                                         rhs=ht1[g - 1][:].rearrange(
                                             "p (b t) -> p b t", t=8)[:, :, 4],
                                         start=False, stop=True,
                                         skip_group_check=True)
                        nc.scalar.activation(dst3, bankB[g][0:64, 0:16], TANH,
                                             bias=biasB[0:64])
                        nc.scalar.activation(dst2a, bankB[g][64:128, 0:16],
                                             TANH, bias=biasB[64:128])
                    v3 = ht1[g][0:64].rearrange("p (b t) -> p b t", t=8)
                    nc.gpsimd.tensor_copy(
                        v3[:, :, 1:8],
                        dst3[:, :, None].to_broadcast([64, B, 7]))
                    v2 = ht1[g][64:128].rearrange("p (b t) -> p b t", t=8)
                    nc.gpsimd.tensor_copy(
                        v2[:, :, 1:4],
                        dst2a[:, :, None].to_broadcast([64, B, 3]))

                if c == 1 and g + 2 < NT:
                    prep_embt(g + 2)
                if c == 5 and g + 1 < NT:
                    u_mms(g + 1)

                # --- chain2 slot 4 (t%8==4), slotted at c==5 ---
                if c == 5:
                    dst2 = hv(ht1[g], 64, 128, 4)
                    # cross from blocks 0,1 at t-1 (col 3)
                    nc.tensor.matmul(out=bankB[g][64:128, 16:32],
                                     lhsT=wX23[:, 64:128],
                                     rhs=ht0[g][:].rearrange(
                                         "p (b t) -> p b t", t=8)[:, :, 3],
                                     start=False, stop=True,
                                     skip_group_check=True)
                    nc.tensor.matmul(out=bankB[g][64:128, 16:32],
                                     lhsT=wB23[64:128, 64:128],
                                     rhs=hv(ht1[g], 64, 128, 0),
                                     start=False, stop=True,
                                     skip_group_check=True)
                    nc.scalar.activation(dst2, bankB[g][64:128, 16:32],
                                         TANH, bias=biasB[64:128])
                    span = min(3, T - t + 1)
                    v2 = ht1[g][64:128].rearrange("p (b t) -> p b t", t=8)
                    nc.gpsimd.tensor_copy(
                        v2[:, :, 5:5 + span],
                        dst2[:, :, None].to_broadcast([64, B, span]))

                # --- projection pacing: 1 unit (2 matmuls) per step ---
                if c == 7 and g >= PROJ_LAG:
                    enqueue_proj(g - PROJ_LAG)
                emit_proj_unit()

            # flush remaining projection work (tiles whose c==7 enqueue
            # never fired: the last PROJ_LAG tiles plus tile NT-1 itself)
            for g in range(NT - PROJ_LAG - 1, NT):
                enqueue_proj(g)
            while proj_q:
                emit_proj_unit()

            if DEBUG_HT:
                for g in range(NT):
                    # dbg row = g*128 + unit_partition, col = token slot b*8+c
                    nc.sync.dma_start(out=d_dbg[g * 128:(g + 1) * 128, 0:128],
                                      in_=ht0[g][:])
                    nc.sync.dma_start(out=d_dbg[g * 128:(g + 1) * 128, 128:256],
                                      in_=ht1[g][:])

    nc.finalize()
    return nc


_NC_CACHE = None
TRACE = False        # set by test harness to capture an NTFF profile
TRACE_KW = {}
LAST_RESULT = None   # BassKernelResults of the most recent run
DEBUG_HT = False     # add a debug output with the recorded h history


def kernel(x, x_sl, embedding, Wi, Wh, bi, bh, Wo):
    global _NC_CACHE, LAST_RESULT
    if _NC_CACHE is None:
        _NC_CACHE = build_program()
    nc = _NC_CACHE

    x = np.asarray(x)
    ids = np.ascontiguousarray(x[:, :T].T).reshape(-1)  # n = t*B + b
    ids_pad = np.zeros(128 * NT, np.int32)
    ids_pad[:B * T] = ids
    ids_dev = np.ascontiguousarray(ids_pad.reshape(NT, 128).T)

    embedding = np.ascontiguousarray(np.asarray(embedding, np.float32))
    Wh16 = np.asarray(Wh, np.float16)
    Wi16 = np.asarray(Wi, np.float16)
    biasv = (np.asarray(bi, np.float32) + np.asarray(bh, np.float32))
    Wo16 = np.asarray(Wo, np.float16)

    whA_h = np.ascontiguousarray(Wh16[0:128, 0:128])
    # [h3;h2] -> [b3|b2] self weights (b3->b2 is masked to zero)
    wB23_h = np.zeros((128, 128), np.float16)
    wB23_h[0:64, 0:64] = Wh16[192:256, 192:256]
    wB23_h[64:128, 0:64] = Wh16[128:192, 192:256]
    wB23_h[64:128, 64:128] = Wh16[128:192, 128:192]
    # [h0;h1] -> [b3|b2] cross weights
    wX23_h = np.ascontiguousarray(
        np.concatenate([Wh16[0:128, 192:256], Wh16[0:128, 128:192]], axis=1))
    wiA_h = np.ascontiguousarray(Wi16[:, 0:128])
    wiB_h = np.ascontiguousarray(
        np.concatenate([Wi16[:, 192:256], Wi16[:, 128:192]], axis=1))
    biasA_h = np.ascontiguousarray(biasv[0:128].reshape(128, 1))
    biasB_h = np.ascontiguousarray(
        np.concatenate([biasv[192:256], biasv[128:192]]).reshape(128, 1))

    in_maps = []
    for cidx in range(NCORES):
        sl = slice(cidx * VS, (cidx + 1) * VS)
        in_maps.append({
            "tok_ids": ids_dev,
            "embedding": embedding,
            "whA": whA_h, "wB23": wB23_h, "wX23": wX23_h,
            "wiA": wiA_h, "wiB": wiB_h,
            "biasA": biasA_h, "biasB": biasB_h,
            "wo0": np.ascontiguousarray(Wo16[0:128, sl]),
            "wo1": np.ascontiguousarray(
                np.concatenate([Wo16[192:256, sl], Wo16[128:192, sl]], axis=0)),
        })

    res = run_bass_kernel_spmd(nc, in_maps, core_ids=list(range(NCORES)),
                               trace=TRACE, **TRACE_KW)
    LAST_RESULT = res
    raw = np.concatenate([r["out"] for r in res.results], axis=1)  # [4096, V]
    out = raw.reshape(NT, B, 8, V).transpose(1, 0, 2, 3).reshape(B, NT * 8, V)
    return out[:, :T].astype(np.float32)
